# revision 1
# baseline (speedup 1.0000x reference)
"""Trainium2 Bass kernel for nn_Attention_30562987278646.

Sharding: 8 cores = 4 batches x 2 head-groups (4 heads each).
Per core: LN(q/k/v slice) -> project with W_in columns of its heads ->
score matrices (cosine + covariance + margin-variance) -> out = S @ f_v
-> partial @ W_out rows. Host sums the 2 head-group partials per batch.

Exact identities used:
 - LN: ln_g folded into W_in on host (W' = diag(g) W); ln_b @ W_in added
   via K=1 accumulating matmuls on device. Device applies (x - mu) * r only.
 - cov term: qc . kc = dots - d*mq*mk -> rank-1 outer product folded as
   extra contraction rows (K=66 matmul: 64 f-rows + means row + ones row).
 - var term: GAMMA=1 and cosine <= 1 mathematically, so
   relu(1 - cos) == 1 - cos; mean_m(1 - cos_nm) = 1 - colsum(cos_nm)/N,
   and colsum(cos_nm)[n] = (sum_m fk_n[:,m]) . fq_n[:,n] -- one tiny matmul.
 - cos_nm == cosine_sim (norms >> 1e-12), computed once.

Everything runs in d-major (transposed) layout so score matrices come out
transposed (S^T[m,n]) and feed the out-stage matmul directly.
"""

import sys
import numpy as np

for _p in ("/opt/trn_rl_repo", "/root/.axon_site/_ro/trn_rl_repo"):
    if _p not in sys.path:
        sys.path.append(_p)

HEADS = 8
DIM_HEAD = 64
LN_EPS = 1e-5
B, N, DIM = 4, 1024, 512
HG = 2                      # head groups (shards along heads)
HPG = HEADS // HG           # heads per group = 4
IG = HPG * DIM_HEAD         # inner dim per group = 256
NT = N // 128               # 8 n-tiles
NC = N // 512               # 2 n-chunks
CC = DIM // 128             # 4 c-chunks


def _build_nc(cos_w: float, cov_w: float, var_w: float):
    import concourse.bass as bass
    import concourse.bacc as bacc
    import concourse.tile as tile
    from concourse import mybir

    f32 = mybir.dt.float32
    f32r = mybir.dt.float32r
    AF = mybir.ActivationFunctionType
    AX = mybir.AxisListType

    def r(ap):
        return ap.bitcast(f32r)

    nc = bacc.Bacc(target_bir_lowering=False, debug=False)
    _lp = nc.allow_low_precision(reason="f32r is 4-byte storage, not low precision")
    _lp.__enter__()

    xin_d = {
        "xq": nc.declare_dram_parameter("xq", [N, DIM], f32, isOutput=False),
        "xk": nc.declare_dram_parameter("xk", [N, DIM], f32, isOutput=False),
        "xv": nc.declare_dram_parameter("xv", [N, DIM], f32, isOutput=False),
    }
    wf = nc.declare_dram_parameter("wf", [DIM, IG], f32, isOutput=False)
    bw = nc.declare_dram_parameter("bw", [64, IG], f32, isOutput=False)
    wo = nc.declare_dram_parameter("wo", [IG, DIM], f32, isOutput=False)
    ident = nc.declare_dram_parameter("ident", [128, 128], f32, isOutput=False)
    sel = nc.declare_dram_parameter("sel", [128, 2], f32, isOutput=False)
    e1 = nc.declare_dram_parameter("e1", [64, 512], f32, isOutput=False)
    eb = nc.declare_dram_parameter("eb", [128, 128], f32, isOutput=False)
    out = nc.declare_dram_parameter("out", [N, DIM], f32, isOutput=True)

    with tile.TileContext(nc) as tc, \
         tc.tile_pool(name="persist", bufs=1) as P, \
         tc.tile_pool(name="stt", bufs=4) as STP, \
         tc.tile_pool(name="small", bufs=6) as SM, \
         tc.tile_pool(name="osb", bufs=8) as OSB, \
         tc.tile_pool(name="psu", bufs=4, space="PSUM") as PSU, \
         tc.tile_pool(name="psc", bufs=2, space="PSUM") as PSC, \
         tc.tile_pool(name="pst", bufs=2, space="PSUM") as PT:

        # ---- constants / weights in SBUF ----
        id_stage = P.tile([128, 128], f32, tag="id_stage", name="id_stage")
        nc.gpsimd.dma_start(out=id_stage, in_=ident[:, :])
        id_sb = P.tile([128, 128], f32, tag="id", name="id_sb")
        nc.scalar.activation(id_sb, id_stage, AF.Copy)
        sel_sb = P.tile([128, 2], f32r, tag="sel", name="sel_sb")
        nc.gpsimd.dma_start(out=sel_sb, in_=sel[:, :].bitcast(f32r))
        e1_sb = P.tile([64, 512], f32r, tag="e1", name="e1_sb")
        nc.gpsimd.dma_start(out=e1_sb, in_=e1[:, :].bitcast(f32r))
        eb_sb = P.tile([128, 128], f32r, tag="eb", name="eb_sb")
        nc.gpsimd.dma_start(out=eb_sb, in_=eb[:, :].bitcast(f32r))
        bw_sb = P.tile([64, IG], f32r, tag="bw", name="bw_sb")
        nc.gpsimd.dma_start(out=bw_sb, in_=bw[:, :].bitcast(f32r))
        eps_sb = P.tile([128, 1], f32, tag="eps", name="eps_sb")
        nc.vector.memset(eps_sb, LN_EPS)
        vw_sb = P.tile([1, 1], f32, tag="vw", name="vw_sb")
        nc.vector.memset(vw_sb, var_w)
        wf_sb = [P.tile([128, IG], f32r, tag=f"wf{c}", name=f"wf{c}") for c in range(CC)]
        for c in range(CC):
            nc.gpsimd.dma_start(out=wf_sb[c], in_=wf[c * 128:(c + 1) * 128, :].bitcast(f32r))
        wo_sb = [P.tile([64, DIM], f32r, tag=f"wo{j}", name=f"wo{j}") for j in range(4)]
        for j in range(4):
            nc.gpsimd.dma_start(out=wo_sb[j], in_=wo[j * 64:(j + 1) * 64, :].bitcast(f32r))

        # ---- persistent activations (projection outputs) ----
        fTq = [P.tile([128, N], f32r, tag=f"fTq{hp}", name=f"fTq{hp}") for hp in range(2)]
        fTk = [P.tile([128, N], f32r, tag=f"fTk{hp}", name=f"fTk{hp}") for hp in range(2)]
        fv_sb = [P.tile([128, IG], f32r, tag=f"fv{mt}", name=f"fv{mt}") for mt in range(NT)]

        # ======== stages A+B under a scoped pool for the xT tiles ========
        with tc.tile_pool(name="xtp", bufs=1) as XT, \
             tc.tile_pool(name="xin", bufs=4) as XIN, \
             tc.tile_pool(name="xdma", bufs=24) as XD:
            xT = {t: [XT.tile([128, N], f32r, tag=f"xT{t}{c}", name=f"xT{t}{c}")
                      for c in range(CC)] for t in ("xq", "xk", "xv")}

            # stage A: load, LN, transpose to c-major
            for t in ("xq", "xk", "xv"):
                for nt in range(NT):
                    xt = XD.tile([128, DIM], f32, tag="xt")
                    nc.gpsimd.dma_start(
                        out=xt, in_=xin_d[t][nt * 128:(nt + 1) * 128, :])
                    stats = SM.tile([128, nc.vector.BN_STATS_DIM], f32,
                                    tag="bns")
                    nc.vector.bn_stats(out=stats, in_=xt)
                    mv = SM.tile([128, nc.vector.BN_AGGR_DIM], f32, tag="bna")
                    nc.vector.bn_aggr(out=mv, in_=stats)
                    std = SM.tile([128, 1], f32, tag="std")
                    nc.scalar.activation(std, mv[:, 1:2], AF.Sqrt, bias=eps_sb)
                    rin = SM.tile([128, 1], f32, tag="rin")
                    nc.vector.reciprocal(rin, std)
                    nmr = SM.tile([128, 1], f32, tag="nmr")
                    nc.vector.tensor_mul(nmr, mv[:, 0:1], rin)
                    nc.vector.tensor_scalar_mul(nmr, nmr, -1.0)
                    zt = XIN.tile([128, DIM], f32, tag="zt")
                    nc.vector.tensor_scalar_mul(zt, xt, rin)
                    xln = XIN.tile([128, DIM], f32, tag="xln")
                    nc.scalar.activation(xln, zt, AF.Identity, bias=nmr)
                    for c in range(CC):
                        pt = PT.tile([128, 128], f32, tag="pt")
                        nc.tensor.transpose(
                            pt, xln[:, c * 128:(c + 1) * 128], id_sb)
                        nc.scalar.activation(
                            xT[t][c][:, nt * 128:(nt + 1) * 128], pt,
                            AF.Copy)

            # stage B: projections (fp32r)
            for tname, fT in (("xq", fTq), ("xk", fTk)):
                for hp in range(2):
                    for ncx in range(NC):
                        pf = PSU.tile([128, 512], f32, tag="big")
                        for c in range(CC):
                            nc.tensor.matmul(
                                pf,
                                r(wf_sb[c][:, hp * 128:(hp + 1) * 128]),
                                r(xT[tname][c][:, ncx * 512:(ncx + 1) * 512]),
                                start=(c == 0), stop=False)
                        nc.tensor.matmul(
                            pf, r(bw_sb[:, hp * 128:(hp + 1) * 128]),
                            r(e1_sb[0:64, 0:512]), start=False, stop=True)
                        nc.vector.tensor_copy(
                            fT[hp][:, ncx * 512:(ncx + 1) * 512], pf)
            for mt in range(NT):
                pf = PSU.tile([128, IG], f32, tag="big")
                for c in range(CC):
                    nc.tensor.matmul(
                        pf, r(xT["xv"][c][:, mt * 128:(mt + 1) * 128]),
                        r(wf_sb[c]), start=(c == 0), stop=False)
                nc.tensor.matmul(pf, r(e1_sb[0:64, 0:128]), r(bw_sb),
                                 start=False, stop=True)
                nc.vector.tensor_copy(fv_sb[mt], pf)

        # ---- stages C-E under a second persist pool (xT memory now free) ----
        with tc.tile_pool(name="p2", bufs=1) as P2:
            fqn = [P2.tile([128, N], f32r, tag=f"fqn{hp}", name=f"fqn{hp}")
                   for hp in range(2)]
            fkn = [P2.tile([128, N], f32r, tag=f"fkn{hp}", name=f"fkn{hp}")
                   for hp in range(2)]
            fqc = [P2.tile([128, N], f32r, tag=f"fqc{hp}", name=f"fqc{hp}")
                   for hp in range(2)]
            # per-head [1,N] stat rows packed at 32-aligned partition bases.
            # Matmul pairs need EQUAL bases on both operands, so each quantity
            # gets its own tile with heads 0-2 at rows 0/32/64, head 3 at row 0
            # of a sibling tile. ONESP provides an all-ones row at each base.
            RP = [P2.tile([97, N], f32r, tag=f"RP{q}", name=f"RP{q}")
                  for q in range(3)]
            RPB = [P2.tile([33, N], f32r, tag=f"RPB{q}", name=f"RPB{q}")
                   for q in range(3)]
            ONESP = P2.tile([97, 128], f32r, tag="ONESP", name="ONESP")
            zst = P2.tile([128, N], f32, tag="zst", name="zst")
            nc.vector.memset(zst, 0.0)
            for q in range(3):
                nc.scalar.activation(RP[q], zst[0:97, :], AF.Copy)
                nc.scalar.activation(RPB[q], zst[0:33, :], AF.Copy)
            ost = P2.tile([97, 128], f32, tag="ost", name="ost")
            nc.vector.memset(ost, 0.0)
            for b in (0, 32, 64):
                nc.vector.memset(ost[b:b + 1, :], 1.0)
            nc.scalar.activation(ONESP, ost, AF.Copy)

            def row(q, h):
                if h < 3:
                    return RP[q][32 * h:32 * h + 1, :]
                return RPB[q][0:1, :]

            def blk(q, h):
                if h < 3:
                    return RP[q][32 * h:32 * h + 32, :]
                return RPB[q][0:32, :]

            def ones_blk(h):
                if h < 3:
                    return ONESP[32 * h:32 * h + 32, 0:128]
                return e1_sb[0:32, 0:128]

            MK, NMQ, VR = 0, 1, 2
            fks = [P2.tile([128, 1], f32r, tag=f"fks{hp}", name=f"fks{hp}")
                   for hp in range(2)]
            oTh = [P2.tile([64, N], f32r, tag=f"oTh{h}", name=f"oTh{h}")
                   for h in range(HPG)]

            # ======== stage C: stats, norms ========
            with tc.tile_pool(name="rows", bufs=1) as RW:
                qsr = [RW.tile([128, N], f32r, tag=f"qsr{hp}", name=f"qsr{hp}")
                       for hp in range(2)]
                ksr = [RW.tile([128, N], f32r, tag=f"ksr{hp}", name=f"ksr{hp}")
                       for hp in range(2)]

                for t_ in qsr + ksr:
                    nc.scalar.activation(t_, zst, AF.Copy)

                def srow(tiles, h):
                    return tiles[h // 2][(h % 2) * 64:(h % 2) * 64 + 1, :]
                # per-head column sums of f and f^2 via M=1 selector matmuls
                for fT, dsq, dsm in ((fTq, qsr, NMQ), (fTk, ksr, MK)):
                    for hp in range(2):
                        sq = STP.tile([128, N], f32r, tag="sq")
                        nc.scalar.activation(sq, fT[hp], AF.Square)
                        for hj in range(2):
                            h = 2 * hp + hj
                            for ncx in range(NC):
                                cs = slice(ncx * 512, (ncx + 1) * 512)
                                p1 = PSU.tile([1, 512], f32, tag="big")
                                nc.tensor.matmul(p1, r(sel_sb[:, hj:hj + 1]),
                                                 r(fT[hp][:, cs]),
                                                 start=True, stop=True)
                                nc.vector.tensor_copy(row(dsm, h)[:, cs], p1)
                                p2 = PSU.tile([1, 512], f32, tag="big")
                                nc.tensor.matmul(p2, r(sel_sb[:, hj:hj + 1]),
                                                 r(sq[:, cs]),
                                                 start=True, stop=True)
                                nc.vector.tensor_copy(srow(dsq, h)[:, cs], p2)
                for h in range(HPG):
                    # qsr: sum(q^2)->cos_w/qn ; ksr: sum(k^2)->1/kn (in place)
                    qr, kr = srow(qsr, h), srow(ksr, h)
                    nc.scalar.activation(qr, qr, AF.Sqrt)
                    nc.vector.reciprocal(qr, qr)
                    nc.vector.tensor_scalar_mul(qr, qr, cos_w)
                    nc.scalar.activation(kr, kr, AF.Sqrt)
                    nc.vector.reciprocal(kr, kr)
                    nc.vector.tensor_scalar_mul(row(MK, h), row(MK, h),
                                                1.0 / DIM_HEAD)
                    nc.vector.tensor_scalar_mul(row(NMQ, h), row(NMQ, h),
                                                -cov_w / DIM_HEAD)
                # broadcast per-head rows across 64 partitions -> fqn/fkn
                for hp in range(2):
                    for ncx in range(NC):
                        cs = slice(ncx * 512, (ncx + 1) * 512)
                        pb = PSU.tile([128, 512], f32, tag="big")
                        nc.tensor.matmul(pb, r(eb_sb),
                                         r(qsr[hp][:, cs]),
                                         start=True, stop=True)
                        nc.vector.tensor_mul(fqn[hp][:, cs],
                                             fTq[hp][:, cs], pb)
                        pb2 = PSU.tile([128, 512], f32, tag="big")
                        nc.tensor.matmul(pb2, r(eb_sb),
                                         r(ksr[hp][:, cs]),
                                         start=True, stop=True)
                        nc.vector.tensor_mul(fkn[hp][:, cs],
                                             fTk[hp][:, cs], pb2)
                    nc.vector.tensor_scalar_mul(fqc[hp], fTq[hp],
                                                cov_w / DIM_HEAD)
                    nc.vector.reduce_sum(fks[hp], fkn[hp], axis=AX.X)
            # var rows: vr = var_w * (1 - colsum(cos)/N)
            for h in range(HPG):
                hp, ds = h // 2, (h % 2) * 64
                for ncx in range(NC):
                    cs = slice(ncx * 512, (ncx + 1) * 512)
                    pv = PSU.tile([1, 512], f32, tag="big")
                    nc.tensor.matmul(
                        pv, r(fks[hp][ds:ds + 64, 0:1]),
                        r(fqn[hp][ds:ds + 64, cs]),
                        start=True, stop=True)
                    nc.scalar.activation(
                        row(VR, h)[:, cs], pv, AF.Identity,
                        bias=vw_sb, scale=-(var_w / (N * cos_w)))

            # ======== stage D: scores + out-stage ========
            di = 0
            for ncx in range(NC):
                cs = slice(ncx * 512, (ncx + 1) * 512)
                for hp in range(2):
                    for hj in range(2):
                        h = 2 * hp + hj
                        ds = (h % 2) * 64
                        po = PSU.tile([64, 512], f32, tag="big")
                        for mt in range(NT):
                            ms = slice(mt * 128, (mt + 1) * 128)
                            pss = PSC.tile([128, 512], f32, tag="pss")
                            nc.tensor.matmul(
                                pss, r(fkn[hp][ds:ds + 64, ms]),
                                r(fqn[hp][ds:ds + 64, cs]),
                                start=True, stop=False)
                            nc.tensor.matmul(
                                pss, r(fTk[hp][ds:ds + 64, ms]),
                                r(fqc[hp][ds:ds + 64, cs]),
                                start=False, stop=False)
                            nc.tensor.matmul(
                                pss, r(blk(MK, h)[:, ms]),
                                r(blk(NMQ, h)[:, cs]),
                                start=False, stop=False)
                            nc.tensor.matmul(
                                pss, r(ones_blk(h)),
                                r(blk(VR, h)[:, cs]),
                                start=False, stop=True)
                            st = STP.tile([128, 512], f32r, tag="st")
                            if di % 2 == 0:
                                nc.vector.tensor_copy(st, pss)
                            else:
                                nc.scalar.activation(st, pss, AF.Copy)
                            di += 1
                            nc.tensor.matmul(
                                po,
                                r(fv_sb[mt][:, h * 64:(h + 1) * 64]),
                                r(st), start=(mt == 0), stop=(mt == NT - 1))
                        nc.scalar.activation(
                            oTh[h][:, ncx * 512:(ncx + 1) * 512], po, AF.Copy)

            # ======== stage E: W_out projection + store ========
            for nt in range(NT):
                pf = PSU.tile([128, 512], f32, tag="big")
                for j in range(4):
                    nc.tensor.matmul(
                        pf, r(oTh[j][:, nt * 128:(nt + 1) * 128]),
                        r(wo_sb[j]), start=(j == 0), stop=(j == 3))
                ob = OSB.tile([128, 512], f32, tag="ob")
                nc.vector.tensor_copy(ob, pf)
                nc.gpsimd.dma_start(out=out[nt * 128:(nt + 1) * 128, :],
                                    in_=ob)

    _lp.__exit__(None, None, None)
    nc.compile()
    return nc


def _prep(q, k, v, ln_g, ln_b, W_in, W_out, b_out, cov_w_raw, var_w_raw):
    q = np.asarray(q, np.float32)
    k = np.asarray(k, np.float32)
    v = np.asarray(v, np.float32)
    ln_g = np.asarray(ln_g, np.float32)
    ln_b = np.asarray(ln_b, np.float32)
    W_in = np.asarray(W_in, np.float32)
    W_out = np.asarray(W_out, np.float32)

    cov_w = float(1.0 / (1.0 + np.exp(-np.float64(cov_w_raw))))
    var_w = float(1.0 / (1.0 + np.exp(-np.float64(var_w_raw))))
    cos_w = 1.0 - cov_w - var_w

    nc = _build_nc(cos_w, cov_w, var_w)

    W_f = (ln_g[:, None] * W_in).astype(np.float32)      # [512, 512]
    bW = (ln_b @ W_in).astype(np.float32)                # [512]
    ident = np.eye(128, dtype=np.float32)
    sel = np.zeros((128, 2), np.float32)
    sel[:64, 0] = 1.0
    sel[64:, 1] = 1.0
    e1 = np.zeros((64, 512), np.float32)
    e1[0, :] = 1.0
    eb = np.zeros((128, 128), np.float32)
    eb[0, :64] = 1.0
    eb[64, 64:] = 1.0

    in_maps = []
    for core in range(8):
        b, g = core // HG, core % HG
        in_maps.append({
            "xq": np.ascontiguousarray(q[b]),
            "xk": np.ascontiguousarray(k[b]),
            "xv": np.ascontiguousarray(v[b]),
            "wf": np.ascontiguousarray(W_f[:, g * IG:(g + 1) * IG]),
            "bw": np.ascontiguousarray(
                np.concatenate([bW[None, g * IG:(g + 1) * IG],
                                np.zeros((63, IG), np.float32)], axis=0)),
            "wo": np.ascontiguousarray(W_out[g * IG:(g + 1) * IG, :]),
            "ident": ident, "sel": sel, "e1": e1, "eb": eb,
        })
    return nc, in_maps


def kernel(q, k, v, ln_g, ln_b, W_in, W_out, b_out, cov_w_raw, var_w_raw):
    from concourse.bass_utils import run_bass_kernel_spmd

    b_out = np.asarray(b_out, np.float32)
    nc, in_maps = _prep(q, k, v, ln_g, ln_b, W_in, W_out, b_out,
                        cov_w_raw, var_w_raw)
    res = run_bass_kernel_spmd(nc, in_maps, list(range(8)))
    parts = [res.results[c]["out"] for c in range(8)]
    out = np.stack([parts[2 * b] + parts[2 * b + 1] + b_out
                    for b in range(B)])
    return out.astype(np.float32)



# revision 6
# speedup vs baseline: 2.0597x; 2.0597x over previous
"""Trainium2 Bass kernel for nn_Attention_30562987278646 — v12.

Sharding: 8 cores = 4 batches x 2 head-groups (4 heads each).

Per core, bf16 data path (2e-2 tolerance):
 A: LN fused into one Pool tensor_scalar (scale+shift) -> bf16 transpose.
 B: projections; W_in bias folded into PSUM->SBUF copies (Act bias col).
    KST[h] = [fTk; fkn], QST[h] = [fqc; fqn] stacked per head.
 C: squares -> sel2 matmul -> norm pairs; fused sqrt/recip; broadcast
    matmul + Pool muls build normalized bottom halves.
    Both rank-1 score terms (mean correction, variance row) are moved to
    the out-stage: NV psum = 3 accumulating matmuls -> rows {nmq, vr};
    wAB[h] = {sum_m Skcol*fv, sum_m fv} via mkcol/ones K=128 matmuls.
 D: score = single K=128 matmul; out-stage accumulates fv^T @ st plus one
    K=2 WAB x NV matmul. Stage E interleaved per n-chunk.

All DMA issue rides the otherwise-idle SP queue (xk loads on Act's HWDGE
to overlap the initial load).
"""

import sys
import numpy as np

for _p in ("/opt/trn_rl_repo", "/root/.axon_site/_ro/trn_rl_repo"):
    if _p not in sys.path:
        sys.path.append(_p)

HEADS = 8
DIM_HEAD = 64
LN_EPS = 1e-5
B, N, DIM = 4, 1024, 512
HG = 2
HPG = HEADS // HG           # heads per group = 4
IG = HPG * DIM_HEAD         # inner dim per group = 256
NT = N // 128               # 8 n-tiles
NC = N // 512               # 2 n-chunks
CC = DIM // 128             # 4 c-chunks


def _build_nc(cos_w: float, cov_w: float, var_w: float):
    import concourse.bass as bass
    import concourse.bacc as bacc
    import concourse.tile as tile
    from concourse import mybir

    f32 = mybir.dt.float32
    f32r = mybir.dt.float32r
    bf16 = mybir.dt.bfloat16
    AF = mybir.ActivationFunctionType
    AX = mybir.AxisListType
    ALU = mybir.AluOpType

    def r(ap):
        return ap.bitcast(f32r)

    nc = bacc.Bacc(target_bir_lowering=False, debug=False)
    _lp = nc.allow_low_precision(reason="2e-2 tolerance; bf16 path validated")
    _lp.__enter__()

    xin_d = {
        "xq": nc.declare_dram_parameter("xq", [N, DIM], f32, isOutput=False),
        "xk": nc.declare_dram_parameter("xk", [N, DIM], f32, isOutput=False),
        "xv": nc.declare_dram_parameter("xv", [N, DIM], f32, isOutput=False),
    }
    wf = nc.declare_dram_parameter("wf", [DIM, IG], bf16, isOutput=False)
    wo = nc.declare_dram_parameter("wo", [IG, DIM], bf16, isOutput=False)
    bwq = nc.declare_dram_parameter("bwq", [64, HPG], f32, isOutput=False)
    bwk = nc.declare_dram_parameter("bwk", [64, HPG], f32, isOutput=False)
    bwv = nc.declare_dram_parameter("bwv", [64, IG], bf16, isOutput=False)
    e1v = nc.declare_dram_parameter("e1v", [64, 128], bf16, isOutput=False)
    ident = nc.declare_dram_parameter("ident", [128, 128], bf16,
                                      isOutput=False)
    sel2 = nc.declare_dram_parameter("sel2", [128, 2], bf16, isOutput=False)
    ocol = nc.declare_dram_parameter("ocol", [128, 1], bf16, isOutput=False)
    qcsel = nc.declare_dram_parameter("qcsel", [64, 2], bf16, isOutput=False)
    vwcol = nc.declare_dram_parameter("vwcol", [2, 1], f32, isOutput=False)
    e2t = nc.declare_dram_parameter("e2t", [66, 128], bf16, isOutput=False)
    e2b = nc.declare_dram_parameter("e2b", [2, 128], bf16, isOutput=False)
    out = nc.declare_dram_parameter("out", [N, DIM], bf16, isOutput=True)

    with tile.TileContext(nc) as tc, \
         tc.tile_pool(name="persist", bufs=1) as P, \
         tc.tile_pool(name="stt", bufs=10) as STP, \
         tc.tile_pool(name="small", bufs=6) as SM, \
         tc.tile_pool(name="osb", bufs=4) as OSB, \
         tc.tile_pool(name="psu", bufs=3, space="PSUM") as PSU, \
         tc.tile_pool(name="pss", bufs=5, space="PSUM") as PSS:

        # ---- weights / constants: SP queue, load order = first use ----
        id_sb = P.tile([128, 128], bf16, tag="id", name="id_sb")
        xts = {t: [] for t in ("xq", "xk", "xv")}
        for t in ("xq", "xk", "xv"):
            for nt in range(NT):
                xts[t].append(P.tile([128, DIM], f32, tag=f"{t}_in{nt}",
                                     name=f"{t}_in{nt}"))

        def load_x(t, lo, hi):
            for nt in range(lo, hi):
                nc.sync.dma_start(
                    out=xts[t][nt],
                    in_=xin_d[t][nt * 128:(nt + 1) * 128, :])

        wf_sb = [P.tile([128, IG], bf16, tag=f"wf{c}", name=f"wf{c}")
                 for c in range(CC)]
        bwq_sb = P.tile([64, HPG], f32, tag="bwq", name="bwq_sb")
        bwk_sb = P.tile([64, HPG], f32, tag="bwk", name="bwk_sb")
        bwv_sb = P.tile([64, IG], bf16, tag="bwv", name="bwv_sb")
        e1v_sb = P.tile([64, 128], bf16, tag="e1v", name="e1v_sb")
        # k loads ride the Pool (SWDGE) queue in parallel with SP's q loads
        for nt in range(NT):
            nc.gpsimd.dma_start(
                out=xts["xk"][nt],
                in_=xin_d["xk"][nt * 128:(nt + 1) * 128, :])
        load_x("xq", 0, 1)
        nc.sync.dma_start(out=id_sb, in_=ident[:, :])
        load_x("xq", 1, 6)
        for c in range(CC):
            nc.sync.dma_start(out=wf_sb[c], in_=wf[c * 128:(c + 1) * 128, :])
        nc.sync.dma_start(out=bwq_sb, in_=bwq[:, :])
        nc.sync.dma_start(out=bwk_sb, in_=bwk[:, :])
        load_x("xq", 6, 8)
        load_x("xv", 0, 8)
        nc.sync.dma_start(out=bwv_sb, in_=bwv[:, :])
        nc.sync.dma_start(out=e1v_sb, in_=e1v[:, :])
        sel2_sb = P.tile([128, 2], bf16, tag="sel2", name="sel2_sb")
        nc.sync.dma_start(out=sel2_sb, in_=sel2[:, :])
        ocol_sb = P.tile([128, 1], bf16, tag="ocol", name="ocol_sb")
        nc.sync.dma_start(out=ocol_sb, in_=ocol[:, :])
        qcsel_sb = P.tile([64, 2], bf16, tag="qcsel", name="qcsel_sb")
        nc.sync.dma_start(out=qcsel_sb, in_=qcsel[:, :])
        vwcol_sb = P.tile([2, 1], f32, tag="vwcol", name="vwcol_sb")
        nc.sync.dma_start(out=vwcol_sb, in_=vwcol[:, :])
        e2t_sb = P.tile([66, 128], bf16, tag="e2t", name="e2t_sb")
        nc.sync.dma_start(out=e2t_sb, in_=e2t[:, :])
        e2b_sb = P.tile([2, 128], bf16, tag="e2b", name="e2b_sb")
        nc.sync.dma_start(out=e2b_sb, in_=e2b[:, :])
        wo_sb = [P.tile([128, DIM], bf16, tag=f"wo{j}", name=f"wo{j}")
                 for j in range(2)]
        for j in range(2):
            nc.sync.dma_start(out=wo_sb[j], in_=wo[j * 128:(j + 1) * 128, :])

        eps_sb = P.tile([128, 1], f32, tag="eps", name="eps_sb")
        nc.vector.memset(eps_sb, LN_EPS)
        # prewarm the Sqrt activation table off the critical path
        warm = P.tile([1, 1], f32, tag="warm", name="warm")
        nc.scalar.activation(warm, eps_sb[0:1, 0:1], AF.Sqrt)

        # ---- persistent activations ----
        KST = [P.tile([128, N], bf16, tag=f"KST{h}", name=f"KST{h}")
               for h in range(HPG)]
        QST = [P.tile([128, N], bf16, tag=f"QST{h}", name=f"QST{h}")
               for h in range(HPG)]
        fv_sb = [P.tile([128, IG], bf16, tag=f"fv{mt}", name=f"fv{mt}")
                 for mt in range(NT)]
        SQ = [P.tile([128, N], bf16, tag=f"SQ{h}", name=f"SQ{h}")
              for h in range(HPG)]
        # row pairs at partition 32h (h<3) / sibling tiles (h=3)
        NVt = P.tile([66, N], bf16, tag="NVt", name="NVt")
        NVb = P.tile([2, N], bf16, tag="NVb", name="NVb")
        WABt = P.tile([66, DIM_HEAD], bf16, tag="WABt", name="WABt")
        WABb = P.tile([2, DIM_HEAD], bf16, tag="WABb", name="WABb")
        NRt = P.tile([66, N], bf16, tag="NRt", name="NRt")
        NRb = P.tile([2, N], bf16, tag="NRb", name="NRb")
        fkscol = [P.tile([128, 2], bf16, tag=f"fks{h}", name=f"fks{h}")
                  for h in range(HPG)]
        pbq0 = P.tile([64, N], bf16, tag="pbq0", name="pbq0")
        pbk0 = P.tile([64, N], bf16, tag="pbk0", name="pbk0")
        mkwb = [[P.tile([128, 2], bf16, tag=f"mkwb{h}_{j}",
                        name=f"mkwb{h}_{j}") for j in range(NT)]
                for h in range(HPG)]
        oT2 = [P.tile([128, N], bf16, tag=f"oT2{j}", name=f"oT2{j}")
               for j in range(2)]

        def row2(tm, tb, h):
            return tm[32 * h:32 * h + 2, :] if h < 3 else tb[0:2, :]

        # ======== stages A+B ========
        with tc.tile_pool(name="xtp", bufs=1) as XT, \
             tc.tile_pool(name="xin", bufs=3) as XIN:
            xT4 = {t: [XT.tile([128, DIM], bf16, tag=f"xT4{t}{nt}",
                               name=f"xT4{t}{nt}") for nt in range(NT)]
                   for t in ("xq", "xk", "xv")}

            def stage_a_nt(t, nt):
                    xt = xts[t][nt]
                    stats = SM.tile([128, nc.vector.BN_STATS_DIM], f32,
                                    tag="bns")
                    nc.vector.bn_stats(out=stats, in_=xt)
                    mv = SM.tile([128, nc.vector.BN_AGGR_DIM], f32, tag="bna")
                    nc.vector.bn_aggr(out=mv, in_=stats)
                    std = SM.tile([128, 1], f32, tag="std")
                    nc.scalar.activation(std, mv[:, 1:2], AF.Sqrt,
                                         bias=eps_sb)
                    rin = SM.tile([128, 1], f32, tag="rin")
                    nc.vector.reciprocal(rin, std)
                    nmr = SM.tile([128, 1], f32, tag="nmr")
                    nc.vector.scalar_tensor_tensor(
                        nmr, mv[:, 0:1], -1.0, rin, ALU.mult, ALU.mult)
                    xln = XIN.tile([128, DIM], bf16, tag="xln")
                    nc.gpsimd.tensor_scalar(
                        xln, xt, rin, nmr, ALU.mult, ALU.add)
                    pt = PSS.tile([128, DIM], bf16, tag="pss")
                    for c in range(CC):
                        nc.tensor.transpose(
                            pt[:, c * 128:(c + 1) * 128],
                            xln[:, c * 128:(c + 1) * 128], id_sb)
                    if nt % 2 == 0:
                        nc.vector.tensor_copy(xT4[t][nt], pt)
                    else:
                        nc.scalar.activation(xT4[t][nt], pt, AF.Copy)

            def stage_b_qk(t):
                dst, bcol, scl = ((QST, bwq_sb, cov_w / DIM_HEAD)
                                  if t == "xq" else (KST, bwk_sb, 1.0))
                for nt in range(NT):
                    ts = slice(nt * 128, (nt + 1) * 128)
                    for hp in range(2):
                        pf = PSU.tile([128, 128], f32, tag="big")
                        for c in range(CC):
                            nc.tensor.matmul(
                                pf,
                                wf_sb[c][:, hp * 128:(hp + 1) * 128],
                                xT4[t][nt][:, c * 128:(c + 1) * 128],
                                start=(c == 0), stop=(c == CC - 1))
                        for hj in range(2):
                            h = 2 * hp + hj
                            src = pf[hj * 64:hj * 64 + 64, 0:128]
                            if (nt + hp) % 2 == 0:
                                nc.scalar.activation(
                                    dst[h][0:64, ts], src, AF.Identity,
                                    bias=bcol[:, h:h + 1], scale=scl)
                            else:
                                nc.vector.tensor_scalar(
                                    dst[h][0:64, ts], src, scl,
                                    bcol[:, h:h + 1], ALU.mult, ALU.add)

            def stage_c_sq(tiles, half):
                # squares into SQ halves (top: fqc^2, bottom: fTk^2)
                for h in range(HPG):
                    nc.gpsimd.tensor_mul(SQ[h][half, :], tiles[h][0:64, :],
                                         tiles[h][0:64, :])

            for nt in range(NT):
                stage_a_nt("xq", nt)
                stage_a_nt("xk", nt)
            stage_b_qk("xq")
            stage_c_sq(QST, slice(0, 64))
            stage_b_qk("xk")
            stage_c_sq(KST, slice(64, 128))
            # norm pairs -> NRt rows; fused sqrt + recip
            nc.vector.memset(NRt, 1.0)
            nc.vector.memset(NRb, 1.0)
            for h in range(HPG):
                nc.gpsimd.memset(fkscol[h], 0.0)
                for j in range(NT):
                    nc.gpsimd.memset(mkwb[h][j][:, 1:2], 1.0)
            for h in range(HPG):
                for ncx in range(NC):
                    cs = slice(ncx * 512, (ncx + 1) * 512)
                    nr2 = PSU.tile([2, 512], f32, tag="big")
                    nc.tensor.matmul(nr2, sel2_sb, SQ[h][:, cs],
                                     start=True, stop=True)
                    nc.vector.tensor_copy(row2(NRt, NRb, h)[:, cs], nr2)
            nc.scalar.activation(NRt, NRt, AF.Sqrt)
            nc.vector.reciprocal(NRt, NRt)
            nc.scalar.activation(NRb, NRb, AF.Sqrt)
            nc.vector.reciprocal(NRb, NRb)

            for nt in range(NT):
                stage_a_nt("xv", nt)
            for mt in range(NT):
                pf = PSU.tile([128, IG], f32, tag="big")
                for c in range(CC):
                    nc.tensor.matmul(
                        pf, xT4["xv"][mt][:, c * 128:(c + 1) * 128],
                        wf_sb[c], start=(c == 0), stop=False)
                nc.tensor.matmul(pf, e1v_sb, bwv_sb, start=False, stop=True)
                nc.scalar.activation(fv_sb[mt], pf, AF.Copy)

        # ======== stage C tail: normalized halves, NV rows ========
        vr_scale = -(var_w / (N * cos_w))
        for h in range(HPG):
            e2s = (e2t_sb[32 * h:32 * h + 2, :] if h < 3 else e2b_sb[0:2, :])
            fkp = [SM.tile([64, 1], f32, tag="fkp", name=f"fkp{h}_{i}")
                   for i in range(NC)]
            for ncx in range(NC):
                cs = slice(ncx * 512, (ncx + 1) * 512)
                pb = PSS.tile([128, 512], f32, tag="pss")
                nc.tensor.matmul(pb, e2s, row2(NRt, NRb, h)[:, cs],
                                 start=True, stop=True)
                nc.vector.tensor_copy(pbq0[:, cs], pb[0:64, 0:512])
                nc.scalar.activation(pbk0[:, cs], pb[64:128, 0:512], AF.Copy)
                nc.gpsimd.tensor_mul(QST[h][64:128, cs],
                                     QST[h][0:64, cs], pbq0[:, cs])
                # fkn chunk sum rides the mul via accum_out (free)
                nc.vector.scalar_tensor_tensor(
                    KST[h][64:128, cs], KST[h][0:64, cs], 1.0,
                    pbk0[:, cs], ALU.bypass, ALU.mult,
                    accum_out=fkp[ncx])
            # fks column (scaled by vr_scale), then NV rows {nmq, vr}
            nc.vector.scalar_tensor_tensor(
                fkscol[h][64:128, 1:2], fkp[0], 1.0, fkp[1],
                ALU.bypass, ALU.add)
            nc.vector.tensor_scalar_mul(fkscol[h][64:128, 1:2],
                                        fkscol[h][64:128, 1:2], vr_scale)
            for ncx in range(NC):
                cs = slice(ncx * 512, (ncx + 1) * 512)
                nv = PSU.tile([2, 512], f32, tag="big")
                nc.tensor.matmul(nv, qcsel_sb, QST[h][0:64, cs],
                                 start=True, stop=False)
                nc.tensor.matmul(nv, fkscol[h][64:128, 0:2],
                                 QST[h][64:128, cs], start=False, stop=True)
                nc.scalar.activation(row2(NVt, NVb, h)[:, cs], nv,
                                     AF.Identity, bias=vwcol_sb[:, 0:1])

        # ======== stage D: scores + out-stage, software-pipelined ========
        # All 8 score matmuls for a (ncx, h) issue back-to-back, then the 8
        # out-stage matmuls: by the time out(mt) issues, st(mt) has long been
        # copied, so the in-order PE queue never stalls on a copy.
        di = 0
        for ncx in range(NC):
            cs = slice(ncx * 512, (ncx + 1) * 512)
            for h in range(HPG):
                hp, ds = h // 2, (h % 2) * 64
                hs = slice(h * 64, (h + 1) * 64)
                po = PSU.tile([64, 512], f32, tag="big")
                if ncx == 0:
                    wab = PSU.tile([2, DIM_HEAD], f32, tag="big")
                    for mt in range(NT):
                        ms = slice(mt * 128, (mt + 1) * 128)
                        mkp = PSS.tile([128, 1], f32, tag="pss")
                        nc.tensor.matmul(mkp, KST[h][0:64, ms],
                                         ocol_sb[0:64, 0:1],
                                         start=True, stop=True)
                        nc.scalar.activation(mkwb[h][mt][:, 0:1], mkp,
                                             AF.Copy)
                sts = []
                for mt in range(NT):
                    ms = slice(mt * 128, (mt + 1) * 128)
                    pss = PSS.tile([128, 512], f32, tag="pss")
                    nc.tensor.matmul(pss, KST[h][:, ms], QST[h][:, cs],
                                     start=True, stop=True)
                    st = STP.tile([128, 512], bf16, tag="st")
                    if di % 2 == 0:
                        nc.scalar.activation(st, pss, AF.Copy)
                    else:
                        nc.vector.tensor_copy(st, pss)
                    di += 1
                    sts.append(st)
                if ncx == 0:
                    for mt in range(NT):
                        nc.tensor.matmul(wab, mkwb[h][mt], fv_sb[mt][:, hs],
                                         start=(mt == 0), stop=(mt == NT - 1))
                for mt in range(NT):
                    nc.tensor.matmul(po, fv_sb[mt][:, hs], sts[mt],
                                     start=(mt == 0), stop=False)
                if ncx == 0:
                    nc.vector.tensor_copy(row2(WABt, WABb, h), wab)
                wabs = (WABt[32 * h:32 * h + 2, :] if h < 3 else WABb[0:2, :])
                nc.tensor.matmul(po, wabs, row2(NVt, NVb, h)[:, cs],
                                 start=False, stop=True)
                nc.scalar.activation(oT2[hp][ds:ds + 64, cs], po, AF.Copy)
            # ---- stage E for this n-chunk ----
            for nt in range(ncx * 4, ncx * 4 + 4):
                pf = PSU.tile([128, 512], f32, tag="big")
                for j in range(2):
                    nc.tensor.matmul(
                        pf, oT2[j][:, nt * 128:(nt + 1) * 128], wo_sb[j],
                        start=(j == 0), stop=(j == 1))
                ob = OSB.tile([128, 512], bf16, tag="ob")
                if nt % 2 == 0:
                    nc.scalar.activation(ob, pf, AF.Copy)
                else:
                    nc.vector.tensor_copy(ob, pf)
                nc.sync.dma_start(out=out[nt * 128:(nt + 1) * 128, :], in_=ob)

    _lp.__exit__(None, None, None)
    nc.compile()
    return nc


def _prep(q, k, v, ln_g, ln_b, W_in, W_out, b_out, cov_w_raw, var_w_raw):
    import ml_dtypes
    bf = ml_dtypes.bfloat16

    q = np.asarray(q, np.float32)
    k = np.asarray(k, np.float32)
    v = np.asarray(v, np.float32)
    ln_g = np.asarray(ln_g, np.float32)
    ln_b = np.asarray(ln_b, np.float32)
    W_in = np.asarray(W_in, np.float32)
    W_out = np.asarray(W_out, np.float32)

    cov_w = float(1.0 / (1.0 + np.exp(-np.float64(cov_w_raw))))
    var_w = float(1.0 / (1.0 + np.exp(-np.float64(var_w_raw))))
    cos_w = 1.0 - cov_w - var_w

    nc = _build_nc(cos_w, cov_w, var_w)

    W_f = (ln_g[:, None] * W_in).astype(np.float32)
    bW = (ln_b @ W_in).astype(np.float32)
    ident = np.eye(128, dtype=np.float32)
    sel2 = np.zeros((128, 2), np.float32)
    sel2[:64, 0] = 1.0
    sel2[64:, 1] = 1.0
    ocol = np.ones((128, 1), np.float32)
    qcsel = np.zeros((64, 2), np.float32)
    qcsel[:, 0] = -1.0 / DIM_HEAD
    vwcol = np.zeros((2, 1), np.float32)
    vwcol[1, 0] = var_w
    e1v = np.zeros((64, 128), np.float32)
    e1v[0, :] = 1.0
    e2t = np.zeros((66, 128), np.float32)
    for h in range(3):
        e2t[32 * h, :64] = cos_w
        e2t[32 * h + 1, 64:] = 1.0
    e2b = np.zeros((2, 128), np.float32)
    e2b[0, :64] = cos_w
    e2b[1, 64:] = 1.0

    in_maps = []
    for core in range(8):
        b, g = core // HG, core % HG
        bWg = bW[g * IG:(g + 1) * IG]
        in_maps.append({
            "xq": np.ascontiguousarray(q[b]),
            "xk": np.ascontiguousarray(k[b]),
            "xv": np.ascontiguousarray(v[b]),
            "wf": np.ascontiguousarray(
                W_f[:, g * IG:(g + 1) * IG]).astype(bf),
            "wo": np.ascontiguousarray(
                W_out[g * IG:(g + 1) * IG, :]).astype(bf),
            "bwq": np.ascontiguousarray(
                bWg.reshape(HPG, 64).T * (cov_w / DIM_HEAD)).astype(
                    np.float32),
            "bwk": np.ascontiguousarray(
                bWg.reshape(HPG, 64).T).astype(np.float32),
            "bwv": np.concatenate(
                [bWg[None, :], np.zeros((63, IG), np.float32)],
                axis=0).astype(bf),
            "e1v": e1v.astype(bf),
            "ident": ident.astype(bf),
            "sel2": sel2.astype(bf),
            "ocol": ocol.astype(bf),
            "qcsel": qcsel.astype(bf),
            "vwcol": vwcol,
            "e2t": e2t.astype(bf),
            "e2b": e2b.astype(bf),
        })
    return nc, in_maps


def kernel(q, k, v, ln_g, ln_b, W_in, W_out, b_out, cov_w_raw, var_w_raw):
    from concourse.bass_utils import run_bass_kernel_spmd

    b_out = np.asarray(b_out, np.float32)
    nc, in_maps = _prep(q, k, v, ln_g, ln_b, W_in, W_out, b_out,
                        cov_w_raw, var_w_raw)
    res = run_bass_kernel_spmd(nc, in_maps, list(range(8)))
    parts = [np.asarray(res.results[c]["out"], np.float32) for c in range(8)]
    out = np.stack([parts[2 * b] + parts[2 * b + 1] + b_out
                    for b in range(B)])
    return out.astype(np.float32)


# revision 7
# speedup vs baseline: 2.1024x; 1.0207x over previous
"""Trainium2 Bass kernel for nn_Attention_30562987278646 — v12.

Sharding: 8 cores = 4 batches x 2 head-groups (4 heads each).

Per core, bf16 data path (2e-2 tolerance):
 A: LN fused into one Pool tensor_scalar (scale+shift) -> bf16 transpose.
 B: projections; W_in bias folded into PSUM->SBUF copies (Act bias col).
    KST[h] = [fTk; fkn], QST[h] = [fqc; fqn] stacked per head.
 C: squares -> sel2 matmul -> norm pairs; fused sqrt/recip; broadcast
    matmul + Pool muls build normalized bottom halves.
    Both rank-1 score terms (mean correction, variance row) are moved to
    the out-stage: NV psum = 3 accumulating matmuls -> rows {nmq, vr};
    wAB[h] = {sum_m Skcol*fv, sum_m fv} via mkcol/ones K=128 matmuls.
 D: score = single K=128 matmul; out-stage accumulates fv^T @ st plus one
    K=2 WAB x NV matmul. Stage E interleaved per n-chunk.

All DMA issue rides the otherwise-idle SP queue (xk loads on Act's HWDGE
to overlap the initial load).
"""

import sys
import numpy as np

for _p in ("/opt/trn_rl_repo", "/root/.axon_site/_ro/trn_rl_repo"):
    if _p not in sys.path:
        sys.path.append(_p)

HEADS = 8
DIM_HEAD = 64
LN_EPS = 1e-5
B, N, DIM = 4, 1024, 512
HG = 2
HPG = HEADS // HG           # heads per group = 4
IG = HPG * DIM_HEAD         # inner dim per group = 256
NT = N // 128               # 8 n-tiles
NC = N // 512               # 2 n-chunks
CC = DIM // 128             # 4 c-chunks


def _build_nc(cos_w: float, cov_w: float, var_w: float):
    import concourse.bass as bass
    import concourse.bacc as bacc
    import concourse.tile as tile
    from concourse import mybir

    f32 = mybir.dt.float32
    f32r = mybir.dt.float32r
    bf16 = mybir.dt.bfloat16
    AF = mybir.ActivationFunctionType
    AX = mybir.AxisListType
    ALU = mybir.AluOpType

    def r(ap):
        return ap.bitcast(f32r)

    nc = bacc.Bacc(target_bir_lowering=False, debug=False)
    _lp = nc.allow_low_precision(reason="2e-2 tolerance; bf16 path validated")
    _lp.__enter__()

    xin_d = {
        "xq": nc.declare_dram_parameter("xq", [N, DIM], f32, isOutput=False),
        "xk": nc.declare_dram_parameter("xk", [N, DIM], f32, isOutput=False),
        "xv": nc.declare_dram_parameter("xv", [N, DIM], f32, isOutput=False),
    }
    wf = nc.declare_dram_parameter("wf", [DIM, IG], bf16, isOutput=False)
    wo = nc.declare_dram_parameter("wo", [IG, DIM], bf16, isOutput=False)
    bwq = nc.declare_dram_parameter("bwq", [64, HPG], f32, isOutput=False)
    bwk = nc.declare_dram_parameter("bwk", [64, HPG], f32, isOutput=False)
    bwv = nc.declare_dram_parameter("bwv", [64, IG], bf16, isOutput=False)
    e1v = nc.declare_dram_parameter("e1v", [64, 128], bf16, isOutput=False)
    ident = nc.declare_dram_parameter("ident", [128, 128], bf16,
                                      isOutput=False)
    sel2 = nc.declare_dram_parameter("sel2", [128, 2], bf16, isOutput=False)
    ocol = nc.declare_dram_parameter("ocol", [128, 1], bf16, isOutput=False)
    qcsel = nc.declare_dram_parameter("qcsel", [64, 2], bf16, isOutput=False)
    vwcol = nc.declare_dram_parameter("vwcol", [2, 1], f32, isOutput=False)
    e2t = nc.declare_dram_parameter("e2t", [66, 128], bf16, isOutput=False)
    e2b = nc.declare_dram_parameter("e2b", [2, 128], bf16, isOutput=False)
    out = nc.declare_dram_parameter("out", [N, DIM], bf16, isOutput=True)

    with tile.TileContext(nc) as tc, \
         tc.tile_pool(name="persist", bufs=1) as P, \
         tc.tile_pool(name="stt", bufs=10) as STP, \
         tc.tile_pool(name="small", bufs=6) as SM, \
         tc.tile_pool(name="osb", bufs=4) as OSB, \
         tc.tile_pool(name="psu", bufs=3, space="PSUM") as PSU, \
         tc.tile_pool(name="pss", bufs=5, space="PSUM") as PSS:

        # ---- weights / constants: SP queue, load order = first use ----
        id_sb = P.tile([128, 128], bf16, tag="id", name="id_sb")
        xts = {t: [] for t in ("xq", "xk", "xv")}
        for t in ("xq", "xk", "xv"):
            for nt in range(NT):
                xts[t].append(P.tile([128, DIM], f32, tag=f"{t}_in{nt}",
                                     name=f"{t}_in{nt}"))

        def load_x(t, lo, hi):
            for nt in range(lo, hi):
                nc.sync.dma_start(
                    out=xts[t][nt],
                    in_=xin_d[t][nt * 128:(nt + 1) * 128, :])

        wf_sb = [P.tile([128, IG], bf16, tag=f"wf{c}", name=f"wf{c}")
                 for c in range(CC)]
        bwq_sb = P.tile([64, HPG], f32, tag="bwq", name="bwq_sb")
        bwk_sb = P.tile([64, HPG], f32, tag="bwk", name="bwk_sb")
        bwv_sb = P.tile([64, IG], bf16, tag="bwv", name="bwv_sb")
        e1v_sb = P.tile([64, 128], bf16, tag="e1v", name="e1v_sb")
        # k loads ride the Pool (SWDGE) queue in parallel with SP's q loads
        for nt in range(NT):
            nc.gpsimd.dma_start(
                out=xts["xk"][nt],
                in_=xin_d["xk"][nt * 128:(nt + 1) * 128, :])
        load_x("xq", 0, 1)
        nc.sync.dma_start(out=id_sb, in_=ident[:, :])
        load_x("xq", 1, 6)
        for c in range(CC):
            nc.sync.dma_start(out=wf_sb[c], in_=wf[c * 128:(c + 1) * 128, :])
        nc.sync.dma_start(out=bwq_sb, in_=bwq[:, :])
        nc.sync.dma_start(out=bwk_sb, in_=bwk[:, :])
        load_x("xq", 6, 8)
        load_x("xv", 0, 8)
        nc.sync.dma_start(out=bwv_sb, in_=bwv[:, :])
        nc.sync.dma_start(out=e1v_sb, in_=e1v[:, :])
        sel2_sb = P.tile([128, 2], bf16, tag="sel2", name="sel2_sb")
        nc.sync.dma_start(out=sel2_sb, in_=sel2[:, :])
        ocol_sb = P.tile([128, 1], bf16, tag="ocol", name="ocol_sb")
        nc.sync.dma_start(out=ocol_sb, in_=ocol[:, :])
        qcsel_sb = P.tile([64, 2], bf16, tag="qcsel", name="qcsel_sb")
        nc.sync.dma_start(out=qcsel_sb, in_=qcsel[:, :])
        vwcol_sb = P.tile([2, 1], f32, tag="vwcol", name="vwcol_sb")
        nc.sync.dma_start(out=vwcol_sb, in_=vwcol[:, :])
        e2t_sb = P.tile([66, 128], bf16, tag="e2t", name="e2t_sb")
        nc.sync.dma_start(out=e2t_sb, in_=e2t[:, :])
        e2b_sb = P.tile([2, 128], bf16, tag="e2b", name="e2b_sb")
        nc.sync.dma_start(out=e2b_sb, in_=e2b[:, :])
        wo_sb = [P.tile([128, DIM], bf16, tag=f"wo{j}", name=f"wo{j}")
                 for j in range(2)]
        for j in range(2):
            nc.sync.dma_start(out=wo_sb[j], in_=wo[j * 128:(j + 1) * 128, :])

        eps_sb = P.tile([128, 1], f32, tag="eps", name="eps_sb")
        nc.vector.memset(eps_sb, LN_EPS)
        # prewarm the Sqrt activation table off the critical path
        warm = P.tile([1, 1], f32, tag="warm", name="warm")
        nc.scalar.activation(warm, eps_sb[0:1, 0:1], AF.Sqrt)

        # ---- persistent activations ----
        KST = [P.tile([128, N], bf16, tag=f"KST{h}", name=f"KST{h}")
               for h in range(HPG)]
        QST = [P.tile([128, N], bf16, tag=f"QST{h}", name=f"QST{h}")
               for h in range(HPG)]
        fv_sb = [P.tile([128, IG], bf16, tag=f"fv{mt}", name=f"fv{mt}")
                 for mt in range(NT)]
        SQ = [P.tile([128, N], bf16, tag=f"SQ{h}", name=f"SQ{h}")
              for h in range(HPG)]
        # row pairs at partition 32h (h<3) / sibling tiles (h=3)
        NVt = P.tile([66, N], bf16, tag="NVt", name="NVt")
        NVb = P.tile([2, N], bf16, tag="NVb", name="NVb")
        WABt = P.tile([66, DIM_HEAD], bf16, tag="WABt", name="WABt")
        WABb = P.tile([2, DIM_HEAD], bf16, tag="WABb", name="WABb")
        NRt = P.tile([66, N], bf16, tag="NRt", name="NRt")
        NRb = P.tile([2, N], bf16, tag="NRb", name="NRb")
        fkscol = [P.tile([128, 2], bf16, tag=f"fks{h}", name=f"fks{h}")
                  for h in range(HPG)]
        pbq0 = P.tile([64, N], bf16, tag="pbq0", name="pbq0")
        pbk0 = P.tile([64, N], bf16, tag="pbk0", name="pbk0")
        mkwb = [[P.tile([128, 2], bf16, tag=f"mkwb{h}_{j}",
                        name=f"mkwb{h}_{j}") for j in range(NT)]
                for h in range(HPG)]
        oT2 = [P.tile([128, N], bf16, tag=f"oT2{j}", name=f"oT2{j}")
               for j in range(2)]

        def row2(tm, tb, h):
            return tm[32 * h:32 * h + 2, :] if h < 3 else tb[0:2, :]

        # ======== stages A+B ========
        with tc.tile_pool(name="xtp", bufs=1) as XT, \
             tc.tile_pool(name="xin", bufs=3) as XIN:
            xT4 = {t: [XT.tile([128, DIM], bf16, tag=f"xT4{t}{nt}",
                               name=f"xT4{t}{nt}") for nt in range(NT)]
                   for t in ("xq", "xk", "xv")}

            def stage_a_nt(t, nt):
                    xt = xts[t][nt]
                    stats = SM.tile([128, nc.vector.BN_STATS_DIM], f32,
                                    tag="bns")
                    nc.vector.bn_stats(out=stats, in_=xt)
                    mv = SM.tile([128, nc.vector.BN_AGGR_DIM], f32, tag="bna")
                    nc.vector.bn_aggr(out=mv, in_=stats)
                    std = SM.tile([128, 1], f32, tag="std")
                    nc.scalar.activation(std, mv[:, 1:2], AF.Sqrt,
                                         bias=eps_sb)
                    rin = SM.tile([128, 1], f32, tag="rin")
                    nc.vector.reciprocal(rin, std)
                    nmr = SM.tile([128, 1], f32, tag="nmr")
                    nc.vector.scalar_tensor_tensor(
                        nmr, mv[:, 0:1], -1.0, rin, ALU.mult, ALU.mult)
                    xln = XIN.tile([128, DIM], bf16, tag="xln")
                    nc.gpsimd.tensor_scalar(
                        xln, xt, rin, nmr, ALU.mult, ALU.add)
                    pt = PSS.tile([128, DIM], bf16, tag="pss")
                    for c in range(CC):
                        nc.tensor.transpose(
                            pt[:, c * 128:(c + 1) * 128],
                            xln[:, c * 128:(c + 1) * 128], id_sb)
                    if nt % 2 == 0:
                        nc.vector.tensor_copy(xT4[t][nt], pt)
                    else:
                        nc.scalar.activation(xT4[t][nt], pt, AF.Copy)

            def stage_b_qk(t):
                dst, bcol, scl = ((QST, bwq_sb, cov_w / DIM_HEAD)
                                  if t == "xq" else (KST, bwk_sb, 1.0))
                for nt in range(NT):
                    ts = slice(nt * 128, (nt + 1) * 128)
                    for hp in range(2):
                        pf = PSU.tile([128, 128], f32, tag="big")
                        for c in range(CC):
                            nc.tensor.matmul(
                                pf,
                                wf_sb[c][:, hp * 128:(hp + 1) * 128],
                                xT4[t][nt][:, c * 128:(c + 1) * 128],
                                start=(c == 0), stop=(c == CC - 1))
                        for hj in range(2):
                            h = 2 * hp + hj
                            src = pf[hj * 64:hj * 64 + 64, 0:128]
                            if (nt + hp) % 2 == 0:
                                nc.scalar.activation(
                                    dst[h][0:64, ts], src, AF.Identity,
                                    bias=bcol[:, h:h + 1], scale=scl)
                            else:
                                nc.vector.tensor_scalar(
                                    dst[h][0:64, ts], src, scl,
                                    bcol[:, h:h + 1], ALU.mult, ALU.add)

            def stage_c_sq(tiles, half):
                # squares into SQ halves (top: fqc^2, bottom: fTk^2)
                for h in range(HPG):
                    nc.gpsimd.tensor_mul(SQ[h][half, :], tiles[h][0:64, :],
                                         tiles[h][0:64, :])

            for nt in range(NT):
                stage_a_nt("xq", nt)
                stage_a_nt("xk", nt)
            stage_b_qk("xq")
            stage_c_sq(QST, slice(0, 64))
            stage_b_qk("xk")
            stage_c_sq(KST, slice(64, 128))
            # norm pairs -> NRt rows; fused sqrt + recip
            nc.vector.memset(NRt, 1.0)
            nc.vector.memset(NRb, 1.0)
            for h in range(HPG):
                nc.gpsimd.memset(fkscol[h], 0.0)
                for j in range(NT):
                    nc.gpsimd.memset(mkwb[h][j][:, 1:2], 1.0)
            for h in range(HPG):
                for ncx in range(NC):
                    cs = slice(ncx * 512, (ncx + 1) * 512)
                    nr2 = PSU.tile([2, 512], f32, tag="big")
                    nc.tensor.matmul(nr2, sel2_sb, SQ[h][:, cs],
                                     start=True, stop=True)
                    nc.vector.tensor_copy(row2(NRt, NRb, h)[:, cs], nr2)
            nc.scalar.activation(NRt, NRt, AF.Sqrt)
            nc.vector.reciprocal(NRt, NRt)
            nc.scalar.activation(NRb, NRb, AF.Sqrt)
            nc.vector.reciprocal(NRb, NRb)

            for nt in range(NT):
                stage_a_nt("xv", nt)
            for mt in range(NT):
                pf = PSU.tile([128, IG], f32, tag="big")
                for c in range(CC):
                    nc.tensor.matmul(
                        pf, xT4["xv"][mt][:, c * 128:(c + 1) * 128],
                        wf_sb[c], start=(c == 0), stop=False)
                nc.tensor.matmul(pf, e1v_sb, bwv_sb, start=False, stop=True)
                nc.scalar.activation(fv_sb[mt], pf, AF.Copy)

        # ======== stage C tail: normalized halves, NV rows ========
        vr_scale = -(var_w / (N * cos_w))
        for h in range(HPG):
            e2s = (e2t_sb[32 * h:32 * h + 2, :] if h < 3 else e2b_sb[0:2, :])
            fkp = [SM.tile([64, 1], f32, tag="fkp", name=f"fkp{h}_{i}")
                   for i in range(NC)]
            for ncx in range(NC):
                cs = slice(ncx * 512, (ncx + 1) * 512)
                pb = PSS.tile([128, 512], f32, tag="pss")
                nc.tensor.matmul(pb, e2s, row2(NRt, NRb, h)[:, cs],
                                 start=True, stop=True)
                nc.vector.tensor_copy(pbq0[:, cs], pb[0:64, 0:512])
                nc.scalar.activation(pbk0[:, cs], pb[64:128, 0:512], AF.Copy)
                nc.gpsimd.tensor_mul(QST[h][64:128, cs],
                                     QST[h][0:64, cs], pbq0[:, cs])
                # fkn chunk sum rides the mul via accum_out (free)
                nc.vector.scalar_tensor_tensor(
                    KST[h][64:128, cs], KST[h][0:64, cs], 1.0,
                    pbk0[:, cs], ALU.bypass, ALU.mult,
                    accum_out=fkp[ncx])
            # fks column (scaled by vr_scale), then NV rows {nmq, vr}
            nc.vector.scalar_tensor_tensor(
                fkscol[h][64:128, 1:2], fkp[0], 1.0, fkp[1],
                ALU.bypass, ALU.add)
            nc.vector.tensor_scalar_mul(fkscol[h][64:128, 1:2],
                                        fkscol[h][64:128, 1:2], vr_scale)
            for ncx in range(NC):
                cs = slice(ncx * 512, (ncx + 1) * 512)
                nv = PSU.tile([2, 512], f32, tag="big")
                nc.tensor.matmul(nv, qcsel_sb, QST[h][0:64, cs],
                                 start=True, stop=False)
                nc.tensor.matmul(nv, fkscol[h][64:128, 0:2],
                                 QST[h][64:128, cs], start=False, stop=True)
                nc.scalar.activation(row2(NVt, NVb, h)[:, cs], nv,
                                     AF.Identity, bias=vwcol_sb[:, 0:1])

        # ======== stage D: scores + out-stage, software-pipelined ========
        # All 8 score matmuls for a (ncx, h) issue back-to-back, then the 8
        # out-stage matmuls: by the time out(mt) issues, st(mt) has long been
        # copied, so the in-order PE queue never stalls on a copy.
        di = 0
        for ncx in range(NC):
            cs = slice(ncx * 512, (ncx + 1) * 512)
            for h in range(HPG):
                hp, ds = h // 2, (h % 2) * 64
                hs = slice(h * 64, (h + 1) * 64)
                po = PSU.tile([64, 512], f32, tag="big")
                if ncx == 0:
                    wab = PSU.tile([2, DIM_HEAD], f32, tag="big")
                    for mt in range(NT):
                        ms = slice(mt * 128, (mt + 1) * 128)
                        mkp = PSS.tile([128, 1], f32, tag="pss")
                        nc.tensor.matmul(mkp, KST[h][0:64, ms],
                                         ocol_sb[0:64, 0:1],
                                         start=True, stop=True)
                        nc.scalar.activation(mkwb[h][mt][:, 0:1], mkp,
                                             AF.Copy)
                sts = []
                for mt in range(NT):
                    ms = slice(mt * 128, (mt + 1) * 128)
                    pss = PSS.tile([128, 512], f32, tag="pss")
                    nc.tensor.matmul(pss, KST[h][:, ms], QST[h][:, cs],
                                     start=True, stop=True)
                    st = STP.tile([128, 512], bf16, tag="st")
                    if di % 8 in (0, 2, 4):
                        nc.scalar.activation(st, pss, AF.Copy)
                    else:
                        nc.vector.tensor_copy(st, pss)
                    di += 1
                    sts.append(st)
                if ncx == 0:
                    for mt in range(NT):
                        nc.tensor.matmul(wab, mkwb[h][mt], fv_sb[mt][:, hs],
                                         start=(mt == 0), stop=(mt == NT - 1))
                for mt in range(NT):
                    nc.tensor.matmul(po, fv_sb[mt][:, hs], sts[mt],
                                     start=(mt == 0), stop=False)
                if ncx == 0:
                    nc.vector.tensor_copy(row2(WABt, WABb, h), wab)
                wabs = (WABt[32 * h:32 * h + 2, :] if h < 3 else WABb[0:2, :])
                nc.tensor.matmul(po, wabs, row2(NVt, NVb, h)[:, cs],
                                 start=False, stop=True)
                nc.scalar.activation(oT2[hp][ds:ds + 64, cs], po, AF.Copy)
            # ---- stage E for this n-chunk ----
            for nt in range(ncx * 4, ncx * 4 + 4):
                pf = PSU.tile([128, 512], f32, tag="big")
                for j in range(2):
                    nc.tensor.matmul(
                        pf, oT2[j][:, nt * 128:(nt + 1) * 128], wo_sb[j],
                        start=(j == 0), stop=(j == 1))
                ob = OSB.tile([128, 512], bf16, tag="ob")
                if nt % 2 == 0:
                    nc.scalar.activation(ob, pf, AF.Copy)
                else:
                    nc.vector.tensor_copy(ob, pf)
                nc.sync.dma_start(out=out[nt * 128:(nt + 1) * 128, :], in_=ob)

    _lp.__exit__(None, None, None)
    nc.compile()
    return nc


def _prep(q, k, v, ln_g, ln_b, W_in, W_out, b_out, cov_w_raw, var_w_raw):
    import ml_dtypes
    bf = ml_dtypes.bfloat16

    q = np.asarray(q, np.float32)
    k = np.asarray(k, np.float32)
    v = np.asarray(v, np.float32)
    ln_g = np.asarray(ln_g, np.float32)
    ln_b = np.asarray(ln_b, np.float32)
    W_in = np.asarray(W_in, np.float32)
    W_out = np.asarray(W_out, np.float32)

    cov_w = float(1.0 / (1.0 + np.exp(-np.float64(cov_w_raw))))
    var_w = float(1.0 / (1.0 + np.exp(-np.float64(var_w_raw))))
    cos_w = 1.0 - cov_w - var_w

    nc = _build_nc(cos_w, cov_w, var_w)

    W_f = (ln_g[:, None] * W_in).astype(np.float32)
    bW = (ln_b @ W_in).astype(np.float32)
    ident = np.eye(128, dtype=np.float32)
    sel2 = np.zeros((128, 2), np.float32)
    sel2[:64, 0] = 1.0
    sel2[64:, 1] = 1.0
    ocol = np.ones((128, 1), np.float32)
    qcsel = np.zeros((64, 2), np.float32)
    qcsel[:, 0] = -1.0 / DIM_HEAD
    vwcol = np.zeros((2, 1), np.float32)
    vwcol[1, 0] = var_w
    e1v = np.zeros((64, 128), np.float32)
    e1v[0, :] = 1.0
    e2t = np.zeros((66, 128), np.float32)
    for h in range(3):
        e2t[32 * h, :64] = cos_w
        e2t[32 * h + 1, 64:] = 1.0
    e2b = np.zeros((2, 128), np.float32)
    e2b[0, :64] = cos_w
    e2b[1, 64:] = 1.0

    in_maps = []
    for core in range(8):
        b, g = core // HG, core % HG
        bWg = bW[g * IG:(g + 1) * IG]
        in_maps.append({
            "xq": np.ascontiguousarray(q[b]),
            "xk": np.ascontiguousarray(k[b]),
            "xv": np.ascontiguousarray(v[b]),
            "wf": np.ascontiguousarray(
                W_f[:, g * IG:(g + 1) * IG]).astype(bf),
            "wo": np.ascontiguousarray(
                W_out[g * IG:(g + 1) * IG, :]).astype(bf),
            "bwq": np.ascontiguousarray(
                bWg.reshape(HPG, 64).T * (cov_w / DIM_HEAD)).astype(
                    np.float32),
            "bwk": np.ascontiguousarray(
                bWg.reshape(HPG, 64).T).astype(np.float32),
            "bwv": np.concatenate(
                [bWg[None, :], np.zeros((63, IG), np.float32)],
                axis=0).astype(bf),
            "e1v": e1v.astype(bf),
            "ident": ident.astype(bf),
            "sel2": sel2.astype(bf),
            "ocol": ocol.astype(bf),
            "qcsel": qcsel.astype(bf),
            "vwcol": vwcol,
            "e2t": e2t.astype(bf),
            "e2b": e2b.astype(bf),
        })
    return nc, in_maps


def kernel(q, k, v, ln_g, ln_b, W_in, W_out, b_out, cov_w_raw, var_w_raw):
    from concourse.bass_utils import run_bass_kernel_spmd

    b_out = np.asarray(b_out, np.float32)
    nc, in_maps = _prep(q, k, v, ln_g, ln_b, W_in, W_out, b_out,
                        cov_w_raw, var_w_raw)
    res = run_bass_kernel_spmd(nc, in_maps, list(range(8)))
    parts = [np.asarray(res.results[c]["out"], np.float32) for c in range(8)]
    out = np.stack([parts[2 * b] + parts[2 * b + 1] + b_out
                    for b in range(B)])
    return out.astype(np.float32)


# revision 8
# speedup vs baseline: 2.5979x; 1.2357x over previous
"""Trainium2 Bass kernel for nn_Attention_30562987278646 — v12.

Sharding: 8 cores = 4 batches x 2 head-groups (4 heads each).

Per core, bf16 data path (2e-2 tolerance):
 A: LN fused into one Pool tensor_scalar (scale+shift) -> bf16 transpose.
 B: projections; W_in bias folded into PSUM->SBUF copies (Act bias col).
    KST[h] = [fTk; fkn], QST[h] = [fqc; fqn] stacked per head.
 C: squares -> sel2 matmul -> norm pairs; fused sqrt/recip; broadcast
    matmul + Pool muls build normalized bottom halves.
    Both rank-1 score terms (mean correction, variance row) are moved to
    the out-stage: NV psum = 3 accumulating matmuls -> rows {nmq, vr};
    wAB[h] = {sum_m Skcol*fv, sum_m fv} via mkcol/ones K=128 matmuls.
 D: score = single K=128 matmul; out-stage accumulates fv^T @ st plus one
    K=2 WAB x NV matmul. Stage E interleaved per n-chunk.

All DMA issue rides the otherwise-idle SP queue (xk loads on Act's HWDGE
to overlap the initial load).
"""

import sys
import numpy as np

for _p in ("/opt/trn_rl_repo", "/root/.axon_site/_ro/trn_rl_repo"):
    if _p not in sys.path:
        sys.path.append(_p)

HEADS = 8
DIM_HEAD = 64
LN_EPS = 1e-5
B, N, DIM = 4, 1024, 512
HG = 2
HPG = HEADS // HG           # heads per group = 4
IG = HPG * DIM_HEAD         # inner dim per group = 256
NT = N // 128               # 8 n-tiles
NC = N // 512               # 2 n-chunks
CC = DIM // 128             # 4 c-chunks


def _build_nc(cos_w: float, cov_w: float, var_w: float):
    import concourse.bass as bass
    import concourse.bacc as bacc
    import concourse.tile as tile
    from concourse import mybir

    f32 = mybir.dt.float32
    f32r = mybir.dt.float32r
    bf16 = mybir.dt.bfloat16
    AF = mybir.ActivationFunctionType
    AX = mybir.AxisListType
    ALU = mybir.AluOpType

    def r(ap):
        return ap.bitcast(f32r)

    nc = bacc.Bacc(target_bir_lowering=False, debug=False)
    _lp = nc.allow_low_precision(reason="2e-2 tolerance; bf16 path validated")
    _lp.__enter__()

    xin_d = {
        "xq": nc.declare_dram_parameter("xq", [N, DIM], f32, isOutput=False),
        "xk": nc.declare_dram_parameter("xk", [N, DIM], f32, isOutput=False),
        "xv": nc.declare_dram_parameter("xv", [N, DIM], f32, isOutput=False),
    }
    wf = nc.declare_dram_parameter("wf", [DIM, IG], bf16, isOutput=False)
    wo = nc.declare_dram_parameter("wo", [IG, DIM], bf16, isOutput=False)
    bwq = nc.declare_dram_parameter("bwq", [64, HPG], f32, isOutput=False)
    bwk = nc.declare_dram_parameter("bwk", [64, HPG], f32, isOutput=False)
    bwv = nc.declare_dram_parameter("bwv", [64, IG], bf16, isOutput=False)
    e1v = nc.declare_dram_parameter("e1v", [64, 128], bf16, isOutput=False)
    ident = nc.declare_dram_parameter("ident", [128, 128], bf16,
                                      isOutput=False)
    sel2 = nc.declare_dram_parameter("sel2", [128, 2], bf16, isOutput=False)
    ocol = nc.declare_dram_parameter("ocol", [128, 1], bf16, isOutput=False)
    oc2 = nc.declare_dram_parameter("oc2", [128, 3], bf16, isOutput=False)
    qcsel = nc.declare_dram_parameter("qcsel", [64, 2], bf16, isOutput=False)
    vwcol = nc.declare_dram_parameter("vwcol", [2, 1], f32, isOutput=False)
    e2t = nc.declare_dram_parameter("e2t", [66, 128], bf16, isOutput=False)
    e2b = nc.declare_dram_parameter("e2b", [2, 128], bf16, isOutput=False)
    out = nc.declare_dram_parameter("out", [N, DIM], bf16, isOutput=True)

    with tile.TileContext(nc) as tc, \
         tc.tile_pool(name="persist", bufs=1) as P, \
         tc.tile_pool(name="stt", bufs=10) as STP, \
         tc.tile_pool(name="small", bufs=6) as SM, \
         tc.tile_pool(name="osb", bufs=4) as OSB, \
         tc.tile_pool(name="psu", bufs=3, space="PSUM") as PSU, \
         tc.tile_pool(name="pss", bufs=5, space="PSUM") as PSS:

        # ---- weights / constants: SP queue, load order = first use ----
        id_sb = P.tile([128, 128], bf16, tag="id", name="id_sb")
        xts = {t: [] for t in ("xq", "xk", "xv")}
        for t in ("xq", "xk", "xv"):
            for nt in range(NT):
                xts[t].append(P.tile([128, DIM], f32, tag=f"{t}_in{nt}",
                                     name=f"{t}_in{nt}"))

        def load_x(t, lo, hi):
            for nt in range(lo, hi):
                nc.sync.dma_start(
                    out=xts[t][nt],
                    in_=xin_d[t][nt * 128:(nt + 1) * 128, :])

        wf_sb = [P.tile([128, IG], bf16, tag=f"wf{c}", name=f"wf{c}")
                 for c in range(CC)]
        bwq_sb = P.tile([64, HPG], f32, tag="bwq", name="bwq_sb")
        bwk_sb = P.tile([64, HPG], f32, tag="bwk", name="bwk_sb")
        bwv_sb = P.tile([64, IG], bf16, tag="bwv", name="bwv_sb")
        e1v_sb = P.tile([64, 128], bf16, tag="e1v", name="e1v_sb")
        # k loads ride the Pool (SWDGE) queue in parallel with SP's q loads
        for nt in range(NT):
            nc.gpsimd.dma_start(
                out=xts["xk"][nt],
                in_=xin_d["xk"][nt * 128:(nt + 1) * 128, :])
        load_x("xq", 0, 1)
        nc.sync.dma_start(out=id_sb, in_=ident[:, :])
        load_x("xq", 1, 6)
        for c in range(CC):
            nc.sync.dma_start(out=wf_sb[c], in_=wf[c * 128:(c + 1) * 128, :])
        nc.sync.dma_start(out=bwq_sb, in_=bwq[:, :])
        nc.sync.dma_start(out=bwk_sb, in_=bwk[:, :])
        load_x("xq", 6, 8)
        load_x("xv", 0, 8)
        nc.sync.dma_start(out=bwv_sb, in_=bwv[:, :])
        nc.sync.dma_start(out=e1v_sb, in_=e1v[:, :])
        sel2_sb = P.tile([128, 2], bf16, tag="sel2", name="sel2_sb")
        nc.sync.dma_start(out=sel2_sb, in_=sel2[:, :])
        ocol_sb = P.tile([128, 1], bf16, tag="ocol", name="ocol_sb")
        nc.sync.dma_start(out=ocol_sb, in_=ocol[:, :])
        oc2_sb = P.tile([128, 3], bf16, tag="oc2", name="oc2_sb")
        nc.sync.dma_start(out=oc2_sb, in_=oc2[:, :])
        qcsel_sb = P.tile([64, 2], bf16, tag="qcsel", name="qcsel_sb")
        nc.sync.dma_start(out=qcsel_sb, in_=qcsel[:, :])
        vwcol_sb = P.tile([2, 1], f32, tag="vwcol", name="vwcol_sb")
        nc.sync.dma_start(out=vwcol_sb, in_=vwcol[:, :])
        e2t_sb = P.tile([66, 128], bf16, tag="e2t", name="e2t_sb")
        nc.sync.dma_start(out=e2t_sb, in_=e2t[:, :])
        e2b_sb = P.tile([2, 128], bf16, tag="e2b", name="e2b_sb")
        nc.sync.dma_start(out=e2b_sb, in_=e2b[:, :])
        wo_sb = [P.tile([128, DIM], bf16, tag=f"wo{j}", name=f"wo{j}")
                 for j in range(2)]
        for j in range(2):
            nc.sync.dma_start(out=wo_sb[j], in_=wo[j * 128:(j + 1) * 128, :])

        eps_sb = P.tile([128, 1], f32, tag="eps", name="eps_sb")
        nc.vector.memset(eps_sb, LN_EPS)
        # prewarm the Sqrt activation table off the critical path
        warm = P.tile([1, 1], f32, tag="warm", name="warm")
        nc.scalar.activation(warm, eps_sb[0:1, 0:1], AF.Sqrt)

        # ---- persistent activations ----
        KST = [P.tile([128, N], bf16, tag=f"KST{h}", name=f"KST{h}")
               for h in range(HPG)]
        QST = [P.tile([128, N], bf16, tag=f"QST{h}", name=f"QST{h}")
               for h in range(HPG)]
        fv_sb = [P.tile([128, IG], bf16, tag=f"fv{mt}", name=f"fv{mt}")
                 for mt in range(NT)]
        SQ = [P.tile([128, N], bf16, tag=f"SQ{h}", name=f"SQ{h}")
              for h in range(HPG)]
        # row pairs at partition 32h (h<3) / sibling tiles (h=3)
        NVt = P.tile([66, N], bf16, tag="NVt", name="NVt")
        NVb = P.tile([2, N], bf16, tag="NVb", name="NVb")
        WABt = P.tile([66, DIM_HEAD], bf16, tag="WABt", name="WABt")
        WABb = P.tile([2, DIM_HEAD], bf16, tag="WABb", name="WABb")
        NRt = P.tile([66, N], bf16, tag="NRt", name="NRt")
        NRb = P.tile([2, N], bf16, tag="NRb", name="NRb")
        fkscol = [P.tile([128, 2], bf16, tag=f"fks{h}", name=f"fks{h}")
                  for h in range(HPG)]
        pbq0 = P.tile([64, N], bf16, tag="pbq0", name="pbq0")
        pbk0 = P.tile([64, N], bf16, tag="pbk0", name="pbk0")
        fkm = [P.tile([128, IG], bf16, tag=f"fkm{mt}", name=f"fkm{mt}")
               for mt in range(NT)]
        fknm = [P.tile([128, IG], bf16, tag=f"fknm{mt}", name=f"fknm{mt}")
                for mt in range(NT)]
        rkncol = [P.tile([128, NT], f32, tag=f"rkc{h}", name=f"rkc{h}")
                  for h in range(HPG)]
        G_sb = [P.tile([128, DIM_HEAD], bf16, tag=f"G{h}", name=f"G{h}")
                for h in range(HPG)]
        oT2 = [P.tile([128, N], bf16, tag=f"oT2{j}", name=f"oT2{j}")
               for j in range(2)]

        def row2(tm, tb, h):
            return tm[32 * h:32 * h + 2, :] if h < 3 else tb[0:2, :]

        # ======== stages A+B ========
        with tc.tile_pool(name="xtp", bufs=1) as XT, \
             tc.tile_pool(name="xin", bufs=3) as XIN:
            xT4 = {t: [XT.tile([128, DIM], bf16, tag=f"xT4{t}{nt}",
                               name=f"xT4{t}{nt}") for nt in range(NT)]
                   for t in ("xq", "xk", "xv")}
            xT4["xk2"] = xT4["xk"]

            def stage_a_nt(t, nt):
                    xt = xts[t][nt]
                    stats = SM.tile([128, nc.vector.BN_STATS_DIM], f32,
                                    tag="bns")
                    nc.vector.bn_stats(out=stats, in_=xt)
                    mv = SM.tile([128, nc.vector.BN_AGGR_DIM], f32, tag="bna")
                    nc.vector.bn_aggr(out=mv, in_=stats)
                    std = SM.tile([128, 1], f32, tag="std")
                    nc.scalar.activation(std, mv[:, 1:2], AF.Sqrt,
                                         bias=eps_sb)
                    rin = SM.tile([128, 1], f32, tag="rin")
                    nc.vector.reciprocal(rin, std)
                    nmr = SM.tile([128, 1], f32, tag="nmr")
                    nc.vector.scalar_tensor_tensor(
                        nmr, mv[:, 0:1], -1.0, rin, ALU.mult, ALU.mult)
                    xln = XIN.tile([128, DIM], bf16, tag="xln")
                    nc.gpsimd.tensor_scalar(
                        xln, xt, rin, nmr, ALU.mult, ALU.add)
                    pt = PSS.tile([128, DIM], bf16, tag="pss")
                    for c in range(CC):
                        nc.tensor.transpose(
                            pt[:, c * 128:(c + 1) * 128],
                            xln[:, c * 128:(c + 1) * 128], id_sb)
                    if nt % 2 == 0:
                        nc.vector.tensor_copy(xT4[t][nt], pt)
                    else:
                        nc.scalar.activation(xT4[t][nt], pt, AF.Copy)

            def stage_b_qk(t):
                dst, bcol, scl = ((QST, bwq_sb, cov_w / DIM_HEAD)
                                  if t == "xq" else (KST, bwk_sb, 1.0))
                for nt in range(NT):
                    ts = slice(nt * 128, (nt + 1) * 128)
                    for hp in range(2):
                        pf = PSU.tile([128, 128], f32, tag="big")
                        for c in range(CC):
                            nc.tensor.matmul(
                                pf,
                                wf_sb[c][:, hp * 128:(hp + 1) * 128],
                                xT4[t][nt][:, c * 128:(c + 1) * 128],
                                start=(c == 0), stop=(c == CC - 1))
                        for hj in range(2):
                            h = 2 * hp + hj
                            src = pf[hj * 64:hj * 64 + 64, 0:128]
                            if (nt + hp) % 2 == 0:
                                nc.scalar.activation(
                                    dst[h][0:64, ts], src, AF.Identity,
                                    bias=bcol[:, h:h + 1], scale=scl)
                            else:
                                nc.vector.tensor_scalar(
                                    dst[h][0:64, ts], src, scl,
                                    bcol[:, h:h + 1], ALU.mult, ALU.add)

            def stage_c_sq(tiles, half):
                # squares into SQ halves (top: fqc^2, bottom: fTk^2)
                for h in range(HPG):
                    nc.gpsimd.tensor_mul(SQ[h][half, :], tiles[h][0:64, :],
                                         tiles[h][0:64, :])

            for nt in range(NT):
                stage_a_nt("xq", nt)
                stage_a_nt("xk", nt)
            stage_b_qk("xq")
            stage_c_sq(QST, slice(0, 64))
            stage_b_qk("xk")
            stage_c_sq(KST, slice(64, 128))
            # norm pairs -> NRt rows; fused sqrt + recip
            nc.vector.memset(NRt, 1.0)
            nc.vector.memset(NRb, 1.0)
            for h in range(HPG):
                nc.gpsimd.memset(fkscol[h], 0.0)
            for h in range(HPG):
                for ncx in range(NC):
                    cs = slice(ncx * 512, (ncx + 1) * 512)
                    nr2 = PSU.tile([2, 512], f32, tag="big")
                    nc.tensor.matmul(nr2, sel2_sb, SQ[h][:, cs],
                                     start=True, stop=True)
                    nc.vector.tensor_copy(row2(NRt, NRb, h)[:, cs], nr2)
            nc.scalar.activation(NRt, NRt, AF.Sqrt)
            nc.vector.reciprocal(NRt, NRt)
            nc.scalar.activation(NRb, NRb, AF.Sqrt)
            nc.vector.reciprocal(NRb, NRb)

            for nt in range(NT):
                stage_a_nt("xv", nt)
            for mt in range(NT):
                pf = PSU.tile([128, IG], f32, tag="big")
                for c in range(CC):
                    nc.tensor.matmul(
                        pf, xT4["xv"][mt][:, c * 128:(c + 1) * 128],
                        wf_sb[c], start=(c == 0), stop=False)
                nc.tensor.matmul(pf, e1v_sb, bwv_sb, start=False, stop=True)
                nc.scalar.activation(fv_sb[mt], pf, AF.Copy)

        # ======== stage C tail: normalized halves, NV rows ========
        vr_scale = -(var_w / (N * cos_w))
        for h in range(HPG):
            e2s = (e2t_sb[32 * h:32 * h + 2, :] if h < 3 else e2b_sb[0:2, :])
            fkp = [SM.tile([64, 1], f32, tag="fkp", name=f"fkp{h}_{i}")
                   for i in range(NC)]
            for ncx in range(NC):
                cs = slice(ncx * 512, (ncx + 1) * 512)
                pb = PSS.tile([128, 512], f32, tag="pss")
                nc.tensor.matmul(pb, e2s, row2(NRt, NRb, h)[:, cs],
                                 start=True, stop=True)
                nc.vector.tensor_copy(pbq0[:, cs], pb[0:64, 0:512])
                nc.scalar.activation(pbk0[:, cs], pb[64:128, 0:512], AF.Copy)
                nc.gpsimd.tensor_mul(QST[h][64:128, cs],
                                     QST[h][0:64, cs], pbq0[:, cs])
                # fkn chunk sum rides the mul via accum_out (free)
                nc.vector.scalar_tensor_tensor(
                    KST[h][64:128, cs], KST[h][0:64, cs], 1.0,
                    pbk0[:, cs], ALU.bypass, ALU.mult,
                    accum_out=fkp[ncx])
            # fks column (scaled by vr_scale), then NV rows {nmq, vr}
            nc.vector.scalar_tensor_tensor(
                fkscol[h][64:128, 1:2], fkp[0], 1.0, fkp[1],
                ALU.bypass, ALU.add)
            nc.vector.tensor_scalar_mul(fkscol[h][64:128, 1:2],
                                        fkscol[h][64:128, 1:2], vr_scale)
            for ncx in range(NC):
                cs = slice(ncx * 512, (ncx + 1) * 512)
                nv = PSU.tile([2, 512], f32, tag="big")
                nc.tensor.matmul(nv, qcsel_sb, QST[h][0:64, cs],
                                 start=True, stop=False)
                nc.tensor.matmul(nv, fkscol[h][64:128, 0:2],
                                 QST[h][64:128, cs], start=False, stop=True)
                nc.scalar.activation(row2(NVt, NVb, h)[:, cs], nv,
                                     AF.Identity, bias=vwcol_sb[:, 0:1])

        # ======== stage G: k-summaries (no N x N scores needed — the
        # bilinear form re-associates: out = (QST^T KST) fv = QST^T (KST fv))
        # token-major k-projections (reuse the v bias row: same bW slice)
        for mt in range(NT):
            pf = PSU.tile([128, IG], f32, tag="big")
            for c in range(CC):
                nc.tensor.matmul(
                    pf, xT4["xk2"][mt][:, c * 128:(c + 1) * 128],
                    wf_sb[c], start=(c == 0), stop=False)
            nc.tensor.matmul(pf, e1v_sb, bwv_sb, start=False, stop=True)
            nc.scalar.activation(fkm[mt], pf, AF.Copy)
        # per-token 1/kn columns from the NR k-rows (staged to base 0)
        for h in range(HPG):
            for mt in range(NT):
                ms = slice(mt * 128, (mt + 1) * 128)
                nrs = SM.tile([2, 128], bf16, tag="nrs")
                nc.vector.tensor_copy(nrs, row2(NRt, NRb, h)[:, ms])
                ptc = PSS.tile([128, 2], bf16, tag="pss")
                nc.tensor.transpose(ptc, nrs, id_sb[0:2, 0:2])
                nc.vector.tensor_copy(rkncol[h][:, mt:mt + 1], ptc[:, 1:2])
            hs = slice(h * 64, (h + 1) * 64)
            for mt in range(NT):
                nc.gpsimd.tensor_scalar(
                    fknm[mt][:, hs], fkm[mt][:, hs],
                    rkncol[h][:, mt:mt + 1], 0.0, ALU.mult, ALU.add)
            graw = PSS.tile([64, DIM_HEAD], f32, tag="pss")
            gnrm = PSS.tile([64, DIM_HEAD], f32, tag="pss")
            for mt in range(NT):
                nc.tensor.matmul(graw, fkm[mt][:, hs], fv_sb[mt][:, hs],
                                 start=(mt == 0), stop=(mt == NT - 1))
            for mt in range(NT):
                nc.tensor.matmul(gnrm, fknm[mt][:, hs], fv_sb[mt][:, hs],
                                 start=(mt == 0), stop=(mt == NT - 1))
            nc.vector.tensor_copy(G_sb[h][0:64, :], graw)
            nc.scalar.activation(G_sb[h][64:128, :], gnrm, AF.Copy)
            # WAB rows: wA = colsum(G_raw), wB = colsum(fv) — one psum pair
            wab = PSU.tile([2, DIM_HEAD], f32, tag="big")
            nc.tensor.matmul(wab, oc2_sb[0:64, 0:2], G_sb[h][0:64, :],
                             start=True, stop=False)
            for mt in range(NT):
                nc.tensor.matmul(wab, oc2_sb[:, 1:3], fv_sb[mt][:, hs],
                                 start=False, stop=(mt == NT - 1))
            nc.vector.tensor_copy(row2(WABt, WABb, h), wab)

        # ======== stage D: tiny out-stage + E ========
        for ncx in range(NC):
            cs = slice(ncx * 512, (ncx + 1) * 512)
            for h in range(HPG):
                hp, ds = h // 2, (h % 2) * 64
                po = PSU.tile([64, 512], f32, tag="big")
                nc.tensor.matmul(po, G_sb[h], QST[h][:, cs],
                                 start=True, stop=False)
                wabs = (WABt[32 * h:32 * h + 2, :] if h < 3 else WABb[0:2, :])
                nc.tensor.matmul(po, wabs, row2(NVt, NVb, h)[:, cs],
                                 start=False, stop=True)
                nc.scalar.activation(oT2[hp][ds:ds + 64, cs], po, AF.Copy)
            for nt in range(ncx * 4, ncx * 4 + 4):
                pf = PSU.tile([128, 512], f32, tag="big")
                for j in range(2):
                    nc.tensor.matmul(
                        pf, oT2[j][:, nt * 128:(nt + 1) * 128], wo_sb[j],
                        start=(j == 0), stop=(j == 1))
                ob = OSB.tile([128, 512], bf16, tag="ob")
                if nt % 2 == 0:
                    nc.scalar.activation(ob, pf, AF.Copy)
                else:
                    nc.vector.tensor_copy(ob, pf)
                nc.sync.dma_start(out=out[nt * 128:(nt + 1) * 128, :], in_=ob)

    _lp.__exit__(None, None, None)
    nc.compile()
    return nc


def _prep(q, k, v, ln_g, ln_b, W_in, W_out, b_out, cov_w_raw, var_w_raw):
    import ml_dtypes
    bf = ml_dtypes.bfloat16

    q = np.asarray(q, np.float32)
    k = np.asarray(k, np.float32)
    v = np.asarray(v, np.float32)
    ln_g = np.asarray(ln_g, np.float32)
    ln_b = np.asarray(ln_b, np.float32)
    W_in = np.asarray(W_in, np.float32)
    W_out = np.asarray(W_out, np.float32)

    cov_w = float(1.0 / (1.0 + np.exp(-np.float64(cov_w_raw))))
    var_w = float(1.0 / (1.0 + np.exp(-np.float64(var_w_raw))))
    cos_w = 1.0 - cov_w - var_w

    nc = _build_nc(cos_w, cov_w, var_w)

    W_f = (ln_g[:, None] * W_in).astype(np.float32)
    bW = (ln_b @ W_in).astype(np.float32)
    ident = np.eye(128, dtype=np.float32)
    sel2 = np.zeros((128, 2), np.float32)
    sel2[:64, 0] = 1.0
    sel2[64:, 1] = 1.0
    ocol = np.ones((128, 1), np.float32)
    oc2 = np.zeros((128, 3), np.float32)
    oc2[:, 0] = 1.0
    oc2[:, 2] = 1.0
    qcsel = np.zeros((64, 2), np.float32)
    qcsel[:, 0] = -1.0 / DIM_HEAD
    vwcol = np.zeros((2, 1), np.float32)
    vwcol[1, 0] = var_w
    e1v = np.zeros((64, 128), np.float32)
    e1v[0, :] = 1.0
    e2t = np.zeros((66, 128), np.float32)
    for h in range(3):
        e2t[32 * h, :64] = cos_w
        e2t[32 * h + 1, 64:] = 1.0
    e2b = np.zeros((2, 128), np.float32)
    e2b[0, :64] = cos_w
    e2b[1, 64:] = 1.0

    in_maps = []
    for core in range(8):
        b, g = core // HG, core % HG
        bWg = bW[g * IG:(g + 1) * IG]
        in_maps.append({
            "xq": np.ascontiguousarray(q[b]),
            "xk": np.ascontiguousarray(k[b]),
            "xv": np.ascontiguousarray(v[b]),
            "wf": np.ascontiguousarray(
                W_f[:, g * IG:(g + 1) * IG]).astype(bf),
            "wo": np.ascontiguousarray(
                W_out[g * IG:(g + 1) * IG, :]).astype(bf),
            "bwq": np.ascontiguousarray(
                bWg.reshape(HPG, 64).T * (cov_w / DIM_HEAD)).astype(
                    np.float32),
            "bwk": np.ascontiguousarray(
                bWg.reshape(HPG, 64).T).astype(np.float32),
            "bwv": np.concatenate(
                [bWg[None, :], np.zeros((63, IG), np.float32)],
                axis=0).astype(bf),
            "e1v": e1v.astype(bf),
            "ident": ident.astype(bf),
            "sel2": sel2.astype(bf),
            "ocol": ocol.astype(bf),
            "oc2": oc2.astype(bf),
            "qcsel": qcsel.astype(bf),
            "vwcol": vwcol,
            "e2t": e2t.astype(bf),
            "e2b": e2b.astype(bf),
        })
    return nc, in_maps


def kernel(q, k, v, ln_g, ln_b, W_in, W_out, b_out, cov_w_raw, var_w_raw):
    from concourse.bass_utils import run_bass_kernel_spmd

    b_out = np.asarray(b_out, np.float32)
    nc, in_maps = _prep(q, k, v, ln_g, ln_b, W_in, W_out, b_out,
                        cov_w_raw, var_w_raw)
    res = run_bass_kernel_spmd(nc, in_maps, list(range(8)))
    parts = [np.asarray(res.results[c]["out"], np.float32) for c in range(8)]
    out = np.stack([parts[2 * b] + parts[2 * b + 1] + b_out
                    for b in range(B)])
    return out.astype(np.float32)


# revision 9
# speedup vs baseline: 2.6137x; 1.0061x over previous
"""Trainium2 Bass kernel for nn_Attention_30562987278646 — v12.

Sharding: 8 cores = 4 batches x 2 head-groups (4 heads each).

Per core, bf16 data path (2e-2 tolerance):
 A: LN fused into one Pool tensor_scalar (scale+shift) -> bf16 transpose.
 B: projections; W_in bias folded into PSUM->SBUF copies (Act bias col).
    KST[h] = [fTk; fkn], QST[h] = [fqc; fqn] stacked per head.
 C: squares -> sel2 matmul -> norm pairs; fused sqrt/recip; broadcast
    matmul + Pool muls build normalized bottom halves.
    Both rank-1 score terms (mean correction, variance row) are moved to
    the out-stage: NV psum = 3 accumulating matmuls -> rows {nmq, vr};
    wAB[h] = {sum_m Skcol*fv, sum_m fv} via mkcol/ones K=128 matmuls.
 D: score = single K=128 matmul; out-stage accumulates fv^T @ st plus one
    K=2 WAB x NV matmul. Stage E interleaved per n-chunk.

All DMA issue rides the otherwise-idle SP queue (xk loads on Act's HWDGE
to overlap the initial load).
"""

import sys
import numpy as np

for _p in ("/opt/trn_rl_repo", "/root/.axon_site/_ro/trn_rl_repo"):
    if _p not in sys.path:
        sys.path.append(_p)

HEADS = 8
DIM_HEAD = 64
LN_EPS = 1e-5
B, N, DIM = 4, 1024, 512
HG = 2
HPG = HEADS // HG           # heads per group = 4
IG = HPG * DIM_HEAD         # inner dim per group = 256
NT = N // 128               # 8 n-tiles
NC = N // 512               # 2 n-chunks
CC = DIM // 128             # 4 c-chunks


def _build_nc(cos_w: float, cov_w: float, var_w: float):
    import concourse.bass as bass
    import concourse.bacc as bacc
    import concourse.tile as tile
    from concourse import mybir

    f32 = mybir.dt.float32
    f32r = mybir.dt.float32r
    bf16 = mybir.dt.bfloat16
    AF = mybir.ActivationFunctionType
    AX = mybir.AxisListType
    ALU = mybir.AluOpType

    def r(ap):
        return ap.bitcast(f32r)

    nc = bacc.Bacc(target_bir_lowering=False, debug=False)
    _lp = nc.allow_low_precision(reason="2e-2 tolerance; bf16 path validated")
    _lp.__enter__()

    xin_d = {
        "xq": nc.declare_dram_parameter("xq", [N, DIM], f32, isOutput=False),
        "xk": nc.declare_dram_parameter("xk", [N, DIM], f32, isOutput=False),
        "xv": nc.declare_dram_parameter("xv", [N, DIM], f32, isOutput=False),
    }
    wf = nc.declare_dram_parameter("wf", [DIM, IG], bf16, isOutput=False)
    wo = nc.declare_dram_parameter("wo", [IG, DIM], bf16, isOutput=False)
    bwq = nc.declare_dram_parameter("bwq", [64, HPG], f32, isOutput=False)
    bwk = nc.declare_dram_parameter("bwk", [64, HPG], f32, isOutput=False)
    bwv = nc.declare_dram_parameter("bwv", [64, IG], bf16, isOutput=False)
    e1v = nc.declare_dram_parameter("e1v", [64, 128], bf16, isOutput=False)
    ident = nc.declare_dram_parameter("ident", [128, 128], bf16,
                                      isOutput=False)
    sel2 = nc.declare_dram_parameter("sel2", [128, 2], bf16, isOutput=False)
    ocol = nc.declare_dram_parameter("ocol", [128, 1], bf16, isOutput=False)
    oc2 = nc.declare_dram_parameter("oc2", [128, 3], bf16, isOutput=False)
    qcsel = nc.declare_dram_parameter("qcsel", [64, 2], bf16, isOutput=False)
    vwcol = nc.declare_dram_parameter("vwcol", [2, 1], f32, isOutput=False)
    e2t = nc.declare_dram_parameter("e2t", [66, 128], bf16, isOutput=False)
    e2b = nc.declare_dram_parameter("e2b", [2, 128], bf16, isOutput=False)
    out = nc.declare_dram_parameter("out", [N, DIM], bf16, isOutput=True)

    with tile.TileContext(nc) as tc, \
         tc.tile_pool(name="persist", bufs=1) as P, \
         tc.tile_pool(name="stt", bufs=10) as STP, \
         tc.tile_pool(name="small", bufs=6) as SM, \
         tc.tile_pool(name="osb", bufs=4) as OSB, \
         tc.tile_pool(name="psu", bufs=3, space="PSUM") as PSU, \
         tc.tile_pool(name="pss", bufs=5, space="PSUM") as PSS:

        # ---- weights / constants: SP queue, load order = first use ----
        id_sb = P.tile([128, 128], bf16, tag="id", name="id_sb")
        xts = {t: [] for t in ("xq", "xk", "xv")}
        for t in ("xq", "xk", "xv"):
            for nt in range(NT):
                xts[t].append(P.tile([128, DIM], f32, tag=f"{t}_in{nt}",
                                     name=f"{t}_in{nt}"))

        def load_x(t, lo, hi):
            for nt in range(lo, hi):
                nc.sync.dma_start(
                    out=xts[t][nt],
                    in_=xin_d[t][nt * 128:(nt + 1) * 128, :])

        wf_sb = [P.tile([128, IG], bf16, tag=f"wf{c}", name=f"wf{c}")
                 for c in range(CC)]
        bwq_sb = P.tile([64, HPG], f32, tag="bwq", name="bwq_sb")
        bwk_sb = P.tile([64, HPG], f32, tag="bwk", name="bwk_sb")
        bwv_sb = P.tile([64, IG], bf16, tag="bwv", name="bwv_sb")
        e1v_sb = P.tile([64, 128], bf16, tag="e1v", name="e1v_sb")
        # k loads ride the Pool (SWDGE) queue in parallel with SP's q loads
        for nt in range(NT):
            nc.gpsimd.dma_start(
                out=xts["xk"][nt],
                in_=xin_d["xk"][nt * 128:(nt + 1) * 128, :])
        load_x("xq", 0, 1)
        nc.sync.dma_start(out=id_sb, in_=ident[:, :])
        load_x("xq", 1, 6)
        for c in range(CC):
            nc.sync.dma_start(out=wf_sb[c], in_=wf[c * 128:(c + 1) * 128, :])
        nc.sync.dma_start(out=bwq_sb, in_=bwq[:, :])
        nc.sync.dma_start(out=bwk_sb, in_=bwk[:, :])
        load_x("xq", 6, 8)
        load_x("xv", 0, 8)
        nc.sync.dma_start(out=bwv_sb, in_=bwv[:, :])
        nc.sync.dma_start(out=e1v_sb, in_=e1v[:, :])
        sel2_sb = P.tile([128, 2], bf16, tag="sel2", name="sel2_sb")
        nc.sync.dma_start(out=sel2_sb, in_=sel2[:, :])
        ocol_sb = P.tile([128, 1], bf16, tag="ocol", name="ocol_sb")
        nc.sync.dma_start(out=ocol_sb, in_=ocol[:, :])
        oc2_sb = P.tile([128, 3], bf16, tag="oc2", name="oc2_sb")
        nc.sync.dma_start(out=oc2_sb, in_=oc2[:, :])
        qcsel_sb = P.tile([64, 2], bf16, tag="qcsel", name="qcsel_sb")
        nc.sync.dma_start(out=qcsel_sb, in_=qcsel[:, :])
        vwcol_sb = P.tile([2, 1], f32, tag="vwcol", name="vwcol_sb")
        nc.sync.dma_start(out=vwcol_sb, in_=vwcol[:, :])
        e2t_sb = P.tile([66, 128], bf16, tag="e2t", name="e2t_sb")
        nc.sync.dma_start(out=e2t_sb, in_=e2t[:, :])
        e2b_sb = P.tile([2, 128], bf16, tag="e2b", name="e2b_sb")
        nc.sync.dma_start(out=e2b_sb, in_=e2b[:, :])
        wo_sb = [P.tile([128, DIM], bf16, tag=f"wo{j}", name=f"wo{j}")
                 for j in range(2)]
        for j in range(2):
            nc.sync.dma_start(out=wo_sb[j], in_=wo[j * 128:(j + 1) * 128, :])

        eps_sb = P.tile([128, 1], f32, tag="eps", name="eps_sb")
        nc.vector.memset(eps_sb, LN_EPS)
        # prewarm the Sqrt activation table off the critical path
        warm = P.tile([1, 1], f32, tag="warm", name="warm")
        nc.scalar.activation(warm, eps_sb[0:1, 0:1], AF.Sqrt)

        # ---- persistent activations ----
        KST = [P.tile([128, N], bf16, tag=f"KST{h}", name=f"KST{h}")
               for h in range(HPG)]
        QST = [P.tile([128, N], bf16, tag=f"QST{h}", name=f"QST{h}")
               for h in range(HPG)]
        fv_sb = [P.tile([128, IG], bf16, tag=f"fv{mt}", name=f"fv{mt}")
                 for mt in range(NT)]
        SQ = [P.tile([128, N], bf16, tag=f"SQ{h}", name=f"SQ{h}")
              for h in range(HPG)]
        # row pairs at partition 32h (h<3) / sibling tiles (h=3)
        NVt = P.tile([66, N], bf16, tag="NVt", name="NVt")
        NVb = P.tile([2, N], bf16, tag="NVb", name="NVb")
        WABt = P.tile([66, DIM_HEAD], bf16, tag="WABt", name="WABt")
        WABb = P.tile([2, DIM_HEAD], bf16, tag="WABb", name="WABb")
        NRt = P.tile([66, N], bf16, tag="NRt", name="NRt")
        NRb = P.tile([2, N], bf16, tag="NRb", name="NRb")
        fkscol = [P.tile([128, 2], bf16, tag=f"fks{h}", name=f"fks{h}")
                  for h in range(HPG)]
        pbq0 = P.tile([64, N], bf16, tag="pbq0", name="pbq0")
        pbk0 = P.tile([64, N], bf16, tag="pbk0", name="pbk0")
        fkm = [P.tile([128, IG], bf16, tag=f"fkm{mt}", name=f"fkm{mt}")
               for mt in range(NT)]
        fknm = [P.tile([128, IG], bf16, tag=f"fknm{mt}", name=f"fknm{mt}")
                for mt in range(NT)]
        rkncol = [P.tile([128, NT], f32, tag=f"rkc{h}", name=f"rkc{h}")
                  for h in range(HPG)]
        G_sb = [P.tile([128, DIM_HEAD], bf16, tag=f"G{h}", name=f"G{h}")
                for h in range(HPG)]
        oT2 = [P.tile([128, N], bf16, tag=f"oT2{j}", name=f"oT2{j}")
               for j in range(2)]

        def row2(tm, tb, h):
            return tm[32 * h:32 * h + 2, :] if h < 3 else tb[0:2, :]

        # ======== stages A+B ========
        with tc.tile_pool(name="xtp", bufs=1) as XT, \
             tc.tile_pool(name="xin", bufs=3) as XIN:
            xT4 = {t: [XT.tile([128, DIM], bf16, tag=f"xT4{t}{nt}",
                               name=f"xT4{t}{nt}") for nt in range(NT)]
                   for t in ("xq", "xk", "xv")}
            xT4["xk2"] = xT4["xk"]

            def stage_a_nt(t, nt):
                    xt = xts[t][nt]
                    stats = SM.tile([128, nc.vector.BN_STATS_DIM], f32,
                                    tag="bns")
                    nc.vector.bn_stats(out=stats, in_=xt)
                    mv = SM.tile([128, nc.vector.BN_AGGR_DIM], f32, tag="bna")
                    nc.vector.bn_aggr(out=mv, in_=stats)
                    std = SM.tile([128, 1], f32, tag="std")
                    nc.scalar.activation(std, mv[:, 1:2], AF.Sqrt,
                                         bias=eps_sb)
                    rin = SM.tile([128, 1], f32, tag="rin")
                    nc.vector.reciprocal(rin, std)
                    nmr = SM.tile([128, 1], f32, tag="nmr")
                    nc.vector.scalar_tensor_tensor(
                        nmr, mv[:, 0:1], -1.0, rin, ALU.mult, ALU.mult)
                    xln = XIN.tile([128, DIM], bf16, tag="xln")
                    nc.gpsimd.tensor_scalar(
                        xln, xt, rin, nmr, ALU.mult, ALU.add)
                    pt = PSS.tile([128, DIM], bf16, tag="pss")
                    for c in range(CC):
                        nc.tensor.transpose(
                            pt[:, c * 128:(c + 1) * 128],
                            xln[:, c * 128:(c + 1) * 128], id_sb)
                    if nt % 2 == 0:
                        nc.vector.tensor_copy(xT4[t][nt], pt)
                    else:
                        nc.scalar.activation(xT4[t][nt], pt, AF.Copy)

            def stage_b_qk(t):
                dst, bcol, scl = ((QST, bwq_sb, cov_w / DIM_HEAD)
                                  if t == "xq" else (KST, bwk_sb, 1.0))
                for nt in range(NT):
                    ts = slice(nt * 128, (nt + 1) * 128)
                    for hp in range(2):
                        pf = PSU.tile([128, 128], f32, tag="big")
                        for c in range(CC):
                            nc.tensor.matmul(
                                pf,
                                wf_sb[c][:, hp * 128:(hp + 1) * 128],
                                xT4[t][nt][:, c * 128:(c + 1) * 128],
                                start=(c == 0), stop=(c == CC - 1))
                        for hj in range(2):
                            h = 2 * hp + hj
                            src = pf[hj * 64:hj * 64 + 64, 0:128]
                            if (nt + hp) % 2 == 0:
                                nc.scalar.activation(
                                    dst[h][0:64, ts], src, AF.Identity,
                                    bias=bcol[:, h:h + 1], scale=scl)
                            else:
                                nc.vector.tensor_scalar(
                                    dst[h][0:64, ts], src, scl,
                                    bcol[:, h:h + 1], ALU.mult, ALU.add)

            def stage_c_sq(tiles, half):
                # squares into SQ halves (top: fqc^2, bottom: fTk^2)
                for h in range(HPG):
                    nc.gpsimd.tensor_mul(SQ[h][half, :], tiles[h][0:64, :],
                                         tiles[h][0:64, :])

            for nt in range(NT):
                stage_a_nt("xq", nt)
                stage_a_nt("xk", nt)
            stage_b_qk("xq")
            stage_c_sq(QST, slice(0, 64))
            stage_b_qk("xk")
            stage_c_sq(KST, slice(64, 128))
            # norm pairs -> NRt rows; fused sqrt + recip
            nc.vector.memset(NRt, 1.0)
            nc.vector.memset(NRb, 1.0)
            for h in range(HPG):
                nc.gpsimd.memset(fkscol[h], 0.0)
            for h in range(HPG):
                for ncx in range(NC):
                    cs = slice(ncx * 512, (ncx + 1) * 512)
                    nr2 = PSU.tile([2, 512], f32, tag="big")
                    nc.tensor.matmul(nr2, sel2_sb, SQ[h][:, cs],
                                     start=True, stop=True)
                    nc.vector.tensor_copy(row2(NRt, NRb, h)[:, cs], nr2)
            nc.scalar.activation(NRt, NRt, AF.Sqrt)
            nc.vector.reciprocal(NRt, NRt)
            nc.scalar.activation(NRb, NRb, AF.Sqrt)
            nc.vector.reciprocal(NRb, NRb)

            for nt in range(NT):
                stage_a_nt("xv", nt)
            for mt in range(NT):
                pf = PSU.tile([128, IG], f32, tag="big")
                for c in range(CC):
                    nc.tensor.matmul(
                        pf, xT4["xv"][mt][:, c * 128:(c + 1) * 128],
                        wf_sb[c], start=(c == 0), stop=False)
                nc.tensor.matmul(pf, e1v_sb, bwv_sb, start=False, stop=True)
                nc.scalar.activation(fv_sb[mt], pf, AF.Copy)

        # ======== stage C tail: normalized halves, NV rows ========
        vr_scale = -(var_w / (N * cos_w))
        for h in range(HPG):
            e2s = (e2t_sb[32 * h:32 * h + 2, :] if h < 3 else e2b_sb[0:2, :])
            fkp = [SM.tile([64, 1], f32, tag="fkp", name=f"fkp{h}_{i}")
                   for i in range(NC)]
            for ncx in range(NC):
                cs = slice(ncx * 512, (ncx + 1) * 512)
                pb = PSS.tile([128, 512], f32, tag="pss")
                nc.tensor.matmul(pb, e2s, row2(NRt, NRb, h)[:, cs],
                                 start=True, stop=True)
                nc.vector.tensor_copy(pbq0[:, cs], pb[0:64, 0:512])
                nc.scalar.activation(pbk0[:, cs], pb[64:128, 0:512], AF.Copy)
                nc.gpsimd.tensor_mul(QST[h][64:128, cs],
                                     QST[h][0:64, cs], pbq0[:, cs])
                # fkn chunk sum rides the mul via accum_out (free)
                nc.vector.scalar_tensor_tensor(
                    KST[h][64:128, cs], KST[h][0:64, cs], 1.0,
                    pbk0[:, cs], ALU.bypass, ALU.mult,
                    accum_out=fkp[ncx])
            # fks column (scaled by vr_scale), then NV rows {nmq, vr}
            nc.vector.scalar_tensor_tensor(
                fkscol[h][64:128, 1:2], fkp[0], 1.0, fkp[1],
                ALU.bypass, ALU.add)
            nc.vector.tensor_scalar_mul(fkscol[h][64:128, 1:2],
                                        fkscol[h][64:128, 1:2], vr_scale)
            for ncx in range(NC):
                cs = slice(ncx * 512, (ncx + 1) * 512)
                nv = PSU.tile([2, 512], f32, tag="big")
                nc.tensor.matmul(nv, qcsel_sb, QST[h][0:64, cs],
                                 start=True, stop=False)
                nc.tensor.matmul(nv, fkscol[h][64:128, 0:2],
                                 QST[h][64:128, cs], start=False, stop=True)
                nc.scalar.activation(row2(NVt, NVb, h)[:, cs], nv,
                                     AF.Identity, bias=vwcol_sb[:, 0:1])

        # ======== stage G: k-summaries (no N x N scores needed — the
        # bilinear form re-associates: out = (QST^T KST) fv = QST^T (KST fv))
        # token-major k-projections (reuse the v bias row: same bW slice)
        for mt in range(NT):
            pf = PSU.tile([128, IG], f32, tag="big")
            for c in range(CC):
                nc.tensor.matmul(
                    pf, xT4["xk2"][mt][:, c * 128:(c + 1) * 128],
                    wf_sb[c], start=(c == 0), stop=False)
            nc.tensor.matmul(pf, e1v_sb, bwv_sb, start=False, stop=True)
            nc.scalar.activation(fkm[mt], pf, AF.Copy)
        # per-token 1/kn columns from the NR k-rows (staged to base 0)
        for h in range(HPG):
            for mt in range(NT):
                ms = slice(mt * 128, (mt + 1) * 128)
                nrs = SM.tile([2, 128], bf16, tag="nrs")
                nc.gpsimd.tensor_copy(nrs, row2(NRt, NRb, h)[:, ms])
                ptc = PSS.tile([128, 2], bf16, tag="pss")
                nc.tensor.transpose(ptc, nrs, id_sb[0:2, 0:2])
                nc.vector.tensor_copy(rkncol[h][:, mt:mt + 1], ptc[:, 1:2])
            hs = slice(h * 64, (h + 1) * 64)
            for mt in range(NT):
                nc.gpsimd.tensor_scalar(
                    fknm[mt][:, hs], fkm[mt][:, hs],
                    rkncol[h][:, mt:mt + 1], 0.0, ALU.mult, ALU.add)
            graw = PSS.tile([64, DIM_HEAD], f32, tag="pss")
            gnrm = PSS.tile([64, DIM_HEAD], f32, tag="pss")
            for mt in range(NT):
                nc.tensor.matmul(graw, fkm[mt][:, hs], fv_sb[mt][:, hs],
                                 start=(mt == 0), stop=(mt == NT - 1))
            for mt in range(NT):
                nc.tensor.matmul(gnrm, fknm[mt][:, hs], fv_sb[mt][:, hs],
                                 start=(mt == 0), stop=(mt == NT - 1))
            nc.vector.tensor_copy(G_sb[h][0:64, :], graw)
            nc.scalar.activation(G_sb[h][64:128, :], gnrm, AF.Copy)
            # WAB rows: wA = colsum(G_raw), wB = colsum(fv) — one psum pair
            wab = PSU.tile([2, DIM_HEAD], f32, tag="big")
            nc.tensor.matmul(wab, oc2_sb[0:64, 0:2], G_sb[h][0:64, :],
                             start=True, stop=False)
            for mt in range(NT):
                nc.tensor.matmul(wab, oc2_sb[:, 1:3], fv_sb[mt][:, hs],
                                 start=False, stop=(mt == NT - 1))
            nc.vector.tensor_copy(row2(WABt, WABb, h), wab)

        # ======== stage D: tiny out-stage + E ========
        for ncx in range(NC):
            cs = slice(ncx * 512, (ncx + 1) * 512)
            for h in range(HPG):
                hp, ds = h // 2, (h % 2) * 64
                po = PSU.tile([64, 512], f32, tag="big")
                nc.tensor.matmul(po, G_sb[h], QST[h][:, cs],
                                 start=True, stop=False)
                wabs = (WABt[32 * h:32 * h + 2, :] if h < 3 else WABb[0:2, :])
                nc.tensor.matmul(po, wabs, row2(NVt, NVb, h)[:, cs],
                                 start=False, stop=True)
                nc.scalar.activation(oT2[hp][ds:ds + 64, cs], po, AF.Copy)
            for nt in range(ncx * 4, ncx * 4 + 4):
                pf = PSU.tile([128, 512], f32, tag="big")
                for j in range(2):
                    nc.tensor.matmul(
                        pf, oT2[j][:, nt * 128:(nt + 1) * 128], wo_sb[j],
                        start=(j == 0), stop=(j == 1))
                ob = OSB.tile([128, 512], bf16, tag="ob")
                if nt % 2 == 0:
                    nc.scalar.activation(ob, pf, AF.Copy)
                else:
                    nc.vector.tensor_copy(ob, pf)
                nc.sync.dma_start(out=out[nt * 128:(nt + 1) * 128, :], in_=ob)

    _lp.__exit__(None, None, None)
    nc.compile()
    return nc


def _prep(q, k, v, ln_g, ln_b, W_in, W_out, b_out, cov_w_raw, var_w_raw):
    import ml_dtypes
    bf = ml_dtypes.bfloat16

    q = np.asarray(q, np.float32)
    k = np.asarray(k, np.float32)
    v = np.asarray(v, np.float32)
    ln_g = np.asarray(ln_g, np.float32)
    ln_b = np.asarray(ln_b, np.float32)
    W_in = np.asarray(W_in, np.float32)
    W_out = np.asarray(W_out, np.float32)

    cov_w = float(1.0 / (1.0 + np.exp(-np.float64(cov_w_raw))))
    var_w = float(1.0 / (1.0 + np.exp(-np.float64(var_w_raw))))
    cos_w = 1.0 - cov_w - var_w

    nc = _build_nc(cos_w, cov_w, var_w)

    W_f = (ln_g[:, None] * W_in).astype(np.float32)
    bW = (ln_b @ W_in).astype(np.float32)
    ident = np.eye(128, dtype=np.float32)
    sel2 = np.zeros((128, 2), np.float32)
    sel2[:64, 0] = 1.0
    sel2[64:, 1] = 1.0
    ocol = np.ones((128, 1), np.float32)
    oc2 = np.zeros((128, 3), np.float32)
    oc2[:, 0] = 1.0
    oc2[:, 2] = 1.0
    qcsel = np.zeros((64, 2), np.float32)
    qcsel[:, 0] = -1.0 / DIM_HEAD
    vwcol = np.zeros((2, 1), np.float32)
    vwcol[1, 0] = var_w
    e1v = np.zeros((64, 128), np.float32)
    e1v[0, :] = 1.0
    e2t = np.zeros((66, 128), np.float32)
    for h in range(3):
        e2t[32 * h, :64] = cos_w
        e2t[32 * h + 1, 64:] = 1.0
    e2b = np.zeros((2, 128), np.float32)
    e2b[0, :64] = cos_w
    e2b[1, 64:] = 1.0

    in_maps = []
    for core in range(8):
        b, g = core // HG, core % HG
        bWg = bW[g * IG:(g + 1) * IG]
        in_maps.append({
            "xq": np.ascontiguousarray(q[b]),
            "xk": np.ascontiguousarray(k[b]),
            "xv": np.ascontiguousarray(v[b]),
            "wf": np.ascontiguousarray(
                W_f[:, g * IG:(g + 1) * IG]).astype(bf),
            "wo": np.ascontiguousarray(
                W_out[g * IG:(g + 1) * IG, :]).astype(bf),
            "bwq": np.ascontiguousarray(
                bWg.reshape(HPG, 64).T * (cov_w / DIM_HEAD)).astype(
                    np.float32),
            "bwk": np.ascontiguousarray(
                bWg.reshape(HPG, 64).T).astype(np.float32),
            "bwv": np.concatenate(
                [bWg[None, :], np.zeros((63, IG), np.float32)],
                axis=0).astype(bf),
            "e1v": e1v.astype(bf),
            "ident": ident.astype(bf),
            "sel2": sel2.astype(bf),
            "ocol": ocol.astype(bf),
            "oc2": oc2.astype(bf),
            "qcsel": qcsel.astype(bf),
            "vwcol": vwcol,
            "e2t": e2t.astype(bf),
            "e2b": e2b.astype(bf),
        })
    return nc, in_maps


def kernel(q, k, v, ln_g, ln_b, W_in, W_out, b_out, cov_w_raw, var_w_raw):
    from concourse.bass_utils import run_bass_kernel_spmd

    b_out = np.asarray(b_out, np.float32)
    nc, in_maps = _prep(q, k, v, ln_g, ln_b, W_in, W_out, b_out,
                        cov_w_raw, var_w_raw)
    res = run_bass_kernel_spmd(nc, in_maps, list(range(8)))
    parts = [np.asarray(res.results[c]["out"], np.float32) for c in range(8)]
    out = np.stack([parts[2 * b] + parts[2 * b + 1] + b_out
                    for b in range(B)])
    return out.astype(np.float32)


# revision 10
# speedup vs baseline: 2.8680x; 1.0973x over previous
"""Trainium2 Bass kernel for nn_Attention_30562987278646 — v12.

Sharding: 8 cores = 4 batches x 2 head-groups (4 heads each).

Per core, bf16 data path (2e-2 tolerance):
 A: LN fused into one Pool tensor_scalar (scale+shift) -> bf16 transpose.
 B: projections; W_in bias folded into PSUM->SBUF copies (Act bias col).
    KST[h] = [fTk; fkn], QST[h] = [fqc; fqn] stacked per head.
 C: squares -> sel2 matmul -> norm pairs; fused sqrt/recip; broadcast
    matmul + Pool muls build normalized bottom halves.
    Both rank-1 score terms (mean correction, variance row) are moved to
    the out-stage: NV psum = 3 accumulating matmuls -> rows {nmq, vr};
    wAB[h] = {sum_m Skcol*fv, sum_m fv} via mkcol/ones K=128 matmuls.
 D: score = single K=128 matmul; out-stage accumulates fv^T @ st plus one
    K=2 WAB x NV matmul. Stage E interleaved per n-chunk.

All DMA issue rides the otherwise-idle SP queue (xk loads on Act's HWDGE
to overlap the initial load).
"""

import sys
import numpy as np

for _p in ("/opt/trn_rl_repo", "/root/.axon_site/_ro/trn_rl_repo"):
    if _p not in sys.path:
        sys.path.append(_p)

HEADS = 8
DIM_HEAD = 64
LN_EPS = 1e-5
B, N, DIM = 4, 1024, 512
HG = 2
HPG = HEADS // HG           # heads per group = 4
IG = HPG * DIM_HEAD         # inner dim per group = 256
NT = N // 128               # 8 n-tiles
NC = N // 512               # 2 n-chunks
CC = DIM // 128             # 4 c-chunks


def _build_nc(cos_w: float, cov_w: float, var_w: float):
    import concourse.bass as bass
    import concourse.bacc as bacc
    import concourse.tile as tile
    from concourse import mybir

    f32 = mybir.dt.float32
    f32r = mybir.dt.float32r
    bf16 = mybir.dt.bfloat16
    AF = mybir.ActivationFunctionType
    AX = mybir.AxisListType
    ALU = mybir.AluOpType

    def r(ap):
        return ap.bitcast(f32r)

    nc = bacc.Bacc(target_bir_lowering=False, debug=False)
    _lp = nc.allow_low_precision(reason="2e-2 tolerance; bf16 path validated")
    _lp.__enter__()

    xin_d = {
        "xq": nc.declare_dram_parameter("xq", [N, DIM], f32, isOutput=False),
        "xk": nc.declare_dram_parameter("xk", [N, DIM], f32, isOutput=False),
        "xv": nc.declare_dram_parameter("xv", [N, DIM], f32, isOutput=False),
    }
    wf = nc.declare_dram_parameter("wf", [DIM, IG], bf16, isOutput=False)
    wo = nc.declare_dram_parameter("wo", [IG, DIM], bf16, isOutput=False)
    bwq = nc.declare_dram_parameter("bwq", [64, HPG], f32, isOutput=False)
    bwk = nc.declare_dram_parameter("bwk", [64, HPG], f32, isOutput=False)
    bwv = nc.declare_dram_parameter("bwv", [64, IG], bf16, isOutput=False)
    e1v = nc.declare_dram_parameter("e1v", [64, 128], bf16, isOutput=False)
    ident = nc.declare_dram_parameter("ident", [128, 128], bf16,
                                      isOutput=False)
    sel2 = nc.declare_dram_parameter("sel2", [128, 2], bf16, isOutput=False)
    ocol = nc.declare_dram_parameter("ocol", [128, 1], bf16, isOutput=False)
    oc2 = nc.declare_dram_parameter("oc2", [128, 3], bf16, isOutput=False)
    qcsel = nc.declare_dram_parameter("qcsel", [64, 2], bf16, isOutput=False)
    vwcol = nc.declare_dram_parameter("vwcol", [2, 1], f32, isOutput=False)
    e2t = nc.declare_dram_parameter("e2t", [66, 128], bf16, isOutput=False)
    e2b = nc.declare_dram_parameter("e2b", [2, 128], bf16, isOutput=False)
    out = nc.declare_dram_parameter("out", [N, DIM], bf16, isOutput=True)

    with tile.TileContext(nc) as tc, \
         tc.tile_pool(name="persist", bufs=1) as P, \
         tc.tile_pool(name="stt", bufs=10) as STP, \
         tc.tile_pool(name="small", bufs=6) as SM, \
         tc.tile_pool(name="osb", bufs=4) as OSB, \
         tc.tile_pool(name="psu", bufs=3, space="PSUM") as PSU, \
         tc.tile_pool(name="pss", bufs=5, space="PSUM") as PSS:

        # ---- weights / constants: SP queue, load order = first use ----
        id_sb = P.tile([128, 128], bf16, tag="id", name="id_sb")
        xts = {t: [] for t in ("xq", "xk", "xv")}
        for t in ("xq", "xk", "xv"):
            for nt in range(NT):
                xts[t].append(P.tile([128, DIM], f32, tag=f"{t}_in{nt}",
                                     name=f"{t}_in{nt}"))

        def load_x(t, lo, hi):
            for nt in range(lo, hi):
                nc.sync.dma_start(
                    out=xts[t][nt],
                    in_=xin_d[t][nt * 128:(nt + 1) * 128, :])

        wf_sb = [P.tile([128, IG], bf16, tag=f"wf{c}", name=f"wf{c}")
                 for c in range(CC)]
        bwq_sb = P.tile([64, HPG], f32, tag="bwq", name="bwq_sb")
        bwk_sb = P.tile([64, HPG], f32, tag="bwk", name="bwk_sb")
        bwv_sb = P.tile([64, IG], bf16, tag="bwv", name="bwv_sb")
        e1v_sb = P.tile([64, 128], bf16, tag="e1v", name="e1v_sb")
        # k loads ride the Pool (SWDGE) queue in parallel with SP's q loads
        for nt in range(NT):
            nc.gpsimd.dma_start(
                out=xts["xk"][nt],
                in_=xin_d["xk"][nt * 128:(nt + 1) * 128, :])
        load_x("xq", 0, 1)
        nc.sync.dma_start(out=id_sb, in_=ident[:, :])
        load_x("xq", 1, 6)
        for c in range(CC):
            nc.sync.dma_start(out=wf_sb[c], in_=wf[c * 128:(c + 1) * 128, :])
        nc.sync.dma_start(out=bwq_sb, in_=bwq[:, :])
        nc.sync.dma_start(out=bwk_sb, in_=bwk[:, :])
        load_x("xq", 6, 8)
        load_x("xv", 0, 8)
        nc.sync.dma_start(out=bwv_sb, in_=bwv[:, :])
        nc.sync.dma_start(out=e1v_sb, in_=e1v[:, :])
        sel2_sb = P.tile([128, 2], bf16, tag="sel2", name="sel2_sb")
        nc.sync.dma_start(out=sel2_sb, in_=sel2[:, :])
        ocol_sb = P.tile([128, 1], bf16, tag="ocol", name="ocol_sb")
        nc.sync.dma_start(out=ocol_sb, in_=ocol[:, :])
        oc2_sb = P.tile([128, 3], bf16, tag="oc2", name="oc2_sb")
        nc.sync.dma_start(out=oc2_sb, in_=oc2[:, :])
        qcsel_sb = P.tile([64, 2], bf16, tag="qcsel", name="qcsel_sb")
        nc.sync.dma_start(out=qcsel_sb, in_=qcsel[:, :])
        vwcol_sb = P.tile([2, 1], f32, tag="vwcol", name="vwcol_sb")
        nc.sync.dma_start(out=vwcol_sb, in_=vwcol[:, :])
        e2t_sb = P.tile([66, 128], bf16, tag="e2t", name="e2t_sb")
        nc.sync.dma_start(out=e2t_sb, in_=e2t[:, :])
        e2b_sb = P.tile([2, 128], bf16, tag="e2b", name="e2b_sb")
        nc.sync.dma_start(out=e2b_sb, in_=e2b[:, :])
        wo_sb = [P.tile([128, DIM], bf16, tag=f"wo{j}", name=f"wo{j}")
                 for j in range(2)]
        for j in range(2):
            nc.sync.dma_start(out=wo_sb[j], in_=wo[j * 128:(j + 1) * 128, :])

        eps_sb = P.tile([128, 1], f32, tag="eps", name="eps_sb")
        nc.vector.memset(eps_sb, LN_EPS)
        # prewarm the Sqrt activation table off the critical path
        warm = P.tile([1, 1], f32, tag="warm", name="warm")
        nc.scalar.activation(warm, eps_sb[0:1, 0:1], AF.Sqrt)

        # ---- persistent activations ----
        QST = [P.tile([128, N], bf16, tag=f"QST{h}", name=f"QST{h}")
               for h in range(HPG)]
        fv_sb = [P.tile([128, IG], bf16, tag=f"fv{mt}", name=f"fv{mt}")
                 for mt in range(NT)]
        SQ = [P.tile([128, N], bf16, tag=f"SQ{h}", name=f"SQ{h}")
              for h in range(HPG)]
        # row pairs at partition 32h (h<3) / sibling tiles (h=3)
        NVt = P.tile([66, N], bf16, tag="NVt", name="NVt")
        NVb = P.tile([2, N], bf16, tag="NVb", name="NVb")
        WABt = P.tile([66, DIM_HEAD], bf16, tag="WABt", name="WABt")
        WABb = P.tile([2, DIM_HEAD], bf16, tag="WABb", name="WABb")
        NRt = P.tile([66, N], bf16, tag="NRt", name="NRt")
        NRb = P.tile([2, N], bf16, tag="NRb", name="NRb")
        fkscol = [P.tile([128, 2], bf16, tag=f"fks{h}", name=f"fks{h}")
                  for h in range(HPG)]
        pbq0 = P.tile([64, N], bf16, tag="pbq0", name="pbq0")
        pbk0 = P.tile([64, N], bf16, tag="pbk0", name="pbk0")
        fkm = [P.tile([128, IG], bf16, tag=f"fkm{mt}", name=f"fkm{mt}")
               for mt in range(NT)]
        fknm = [P.tile([128, IG], bf16, tag=f"fknm{mt}", name=f"fknm{mt}")
                for mt in range(NT)]
        rkncol = [P.tile([128, NT], f32, tag=f"rkc{h}", name=f"rkc{h}")
                  for h in range(HPG)]
        G_sb = [P.tile([128, DIM_HEAD], bf16, tag=f"G{h}", name=f"G{h}")
                for h in range(HPG)]
        oT2 = [P.tile([128, N], bf16, tag=f"oT2{j}", name=f"oT2{j}")
               for j in range(2)]

        def row2(tm, tb, h):
            return tm[32 * h:32 * h + 2, :] if h < 3 else tb[0:2, :]

        # ======== stages A+B ========
        with tc.tile_pool(name="xtp", bufs=1) as XT, \
             tc.tile_pool(name="xin", bufs=3) as XIN:
            xT4 = {t: [XT.tile([128, DIM], bf16, tag=f"xT4{t}{nt}",
                               name=f"xT4{t}{nt}") for nt in range(NT)]
                   for t in ("xq", "xk", "xv")}
            xT4["xk2"] = xT4["xk"]

            def stage_a_nt(t, nt):
                    xt = xts[t][nt]
                    stats = SM.tile([128, nc.vector.BN_STATS_DIM], f32,
                                    tag="bns")
                    nc.vector.bn_stats(out=stats, in_=xt)
                    mv = SM.tile([128, nc.vector.BN_AGGR_DIM], f32, tag="bna")
                    nc.vector.bn_aggr(out=mv, in_=stats)
                    std = SM.tile([128, 1], f32, tag="std")
                    nc.scalar.activation(std, mv[:, 1:2], AF.Sqrt,
                                         bias=eps_sb)
                    rin = SM.tile([128, 1], f32, tag="rin")
                    nc.vector.reciprocal(rin, std)
                    nmr = SM.tile([128, 1], f32, tag="nmr")
                    nc.vector.scalar_tensor_tensor(
                        nmr, mv[:, 0:1], -1.0, rin, ALU.mult, ALU.mult)
                    xln = XIN.tile([128, DIM], bf16, tag="xln")
                    nc.gpsimd.tensor_scalar(
                        xln, xt, rin, nmr, ALU.mult, ALU.add)
                    pt = PSS.tile([128, DIM], bf16, tag="pss")
                    for c in range(CC):
                        nc.tensor.transpose(
                            pt[:, c * 128:(c + 1) * 128],
                            xln[:, c * 128:(c + 1) * 128], id_sb)
                    if nt % 2 == 0:
                        nc.vector.tensor_copy(xT4[t][nt], pt)
                    else:
                        nc.scalar.activation(xT4[t][nt], pt, AF.Copy)

            def stage_b_qk(t):
                dst, bcol, scl = (QST, bwq_sb, cov_w / DIM_HEAD)
                for nt in range(NT):
                    ts = slice(nt * 128, (nt + 1) * 128)
                    for hp in range(2):
                        pf = PSU.tile([128, 128], f32, tag="big")
                        for c in range(CC):
                            nc.tensor.matmul(
                                pf,
                                wf_sb[c][:, hp * 128:(hp + 1) * 128],
                                xT4[t][nt][:, c * 128:(c + 1) * 128],
                                start=(c == 0), stop=(c == CC - 1))
                        for hj in range(2):
                            h = 2 * hp + hj
                            src = pf[hj * 64:hj * 64 + 64, 0:128]
                            if (nt + hp) % 2 == 0:
                                nc.scalar.activation(
                                    dst[h][0:64, ts], src, AF.Identity,
                                    bias=bcol[:, h:h + 1], scale=scl)
                            else:
                                nc.vector.tensor_scalar(
                                    dst[h][0:64, ts], src, scl,
                                    bcol[:, h:h + 1], ALU.mult, ALU.add)

            def stage_c_sq(tiles, half):
                # squares into SQ halves (top: fqc^2, bottom: fTk^2)
                for h in range(HPG):
                    nc.gpsimd.tensor_mul(SQ[h][half, :], tiles[h][0:64, :],
                                         tiles[h][0:64, :])

            for nt in range(NT):
                stage_a_nt("xq", nt)
                stage_a_nt("xk", nt)
            stage_b_qk("xq")
            stage_c_sq(QST, slice(0, 64))
            # token-major k projections (B_k d-major path deleted)
            for mt in range(NT):
                pf = PSU.tile([128, IG], f32, tag="big")
                for c in range(CC):
                    nc.tensor.matmul(
                        pf, xT4["xk"][mt][:, c * 128:(c + 1) * 128],
                        wf_sb[c], start=(c == 0), stop=False)
                nc.tensor.matmul(pf, e1v_sb, bwv_sb, start=False, stop=True)
                nc.scalar.activation(fkm[mt], pf, AF.Copy)
            # per-token 1/kn columns: square-accum + batched sqrt/recip
            for h in range(HPG):
                hsl = slice(h * 64, (h + 1) * 64)
                for mt in range(NT):
                    scr = SM.tile([128, DIM_HEAD], bf16, tag="scr")
                    nc.vector.scalar_tensor_tensor(
                        scr, fkm[mt][:, hsl], 1.0, fkm[mt][:, hsl],
                        ALU.bypass, ALU.mult,
                        accum_out=rkncol[h][:, mt:mt + 1])
                nc.scalar.activation(rkncol[h], rkncol[h], AF.Sqrt)
                nc.vector.reciprocal(rkncol[h], rkncol[h])
            # norm pairs -> NRt rows; fused sqrt + recip
            nc.vector.memset(NRt, 1.0)
            nc.vector.memset(NRb, 1.0)
            for h in range(HPG):
                nc.gpsimd.memset(fkscol[h], 0.0)
            for h in range(HPG):
                for ncx in range(NC):
                    cs = slice(ncx * 512, (ncx + 1) * 512)
                    nr2 = PSU.tile([1, 512], f32, tag="big")
                    nc.tensor.matmul(nr2, sel2_sb[0:64, 0:1],
                                     SQ[h][0:64, cs], start=True, stop=True)
                    nc.vector.tensor_copy(row2(NRt, NRb, h)[0:1, cs], nr2)
            nc.scalar.activation(NRt, NRt, AF.Sqrt)
            nc.vector.reciprocal(NRt, NRt)
            nc.scalar.activation(NRb, NRb, AF.Sqrt)
            nc.vector.reciprocal(NRb, NRb)

            for nt in range(NT):
                stage_a_nt("xv", nt)
            for mt in range(NT):
                pf = PSU.tile([128, IG], f32, tag="big")
                for c in range(CC):
                    nc.tensor.matmul(
                        pf, xT4["xv"][mt][:, c * 128:(c + 1) * 128],
                        wf_sb[c], start=(c == 0), stop=False)
                nc.tensor.matmul(pf, e1v_sb, bwv_sb, start=False, stop=True)
                nc.scalar.activation(fv_sb[mt], pf, AF.Copy)
            for h in range(HPG):
                hsl = slice(h * 64, (h + 1) * 64)
                for mt in range(NT):
                    nc.gpsimd.tensor_scalar(
                        fknm[mt][:, hsl], fkm[mt][:, hsl],
                        rkncol[h][:, mt:mt + 1], 0.0, ALU.mult, ALU.add)

        # ======== stage C tail: normalized halves, NV rows ========
        vr_scale = -(var_w / (N * cos_w))
        for h in range(HPG):
            e2s = (e2t_sb[32 * h:32 * h + 2, :] if h < 3 else e2b_sb[0:2, :])
            for ncx in range(NC):
                cs = slice(ncx * 512, (ncx + 1) * 512)
                pb = PSS.tile([128, 512], f32, tag="pss")
                nc.tensor.matmul(pb, e2s, row2(NRt, NRb, h)[:, cs],
                                 start=True, stop=True)
                nc.vector.tensor_copy(pbq0[:, cs], pb[0:64, 0:512])
                nc.gpsimd.tensor_mul(QST[h][64:128, cs],
                                     QST[h][0:64, cs], pbq0[:, cs])
            # fks from fknm: row accumulate -> transpose -> scaled column
            hsl = slice(h * 64, (h + 1) * 64)
            fkr = PSS.tile([1, DIM_HEAD], f32, tag="pss")
            for mt in range(NT):
                nc.tensor.matmul(fkr, ocol_sb[:, 0:1], fknm[mt][:, hsl],
                                 start=(mt == 0), stop=(mt == NT - 1))
            fkrs = SM.tile([1, DIM_HEAD], bf16, tag="fkrs")
            nc.vector.tensor_copy(fkrs, fkr)
            fkc = PSS.tile([DIM_HEAD, 1], bf16, tag="pss")
            nc.tensor.transpose(fkc, fkrs, id_sb[0:1, 0:1])
            nc.scalar.activation(fkscol[h][64:128, 1:2], fkc, AF.Identity,
                                 scale=vr_scale)
            for ncx in range(NC):
                cs = slice(ncx * 512, (ncx + 1) * 512)
                nv = PSU.tile([2, 512], f32, tag="big")
                nc.tensor.matmul(nv, qcsel_sb, QST[h][0:64, cs],
                                 start=True, stop=False)
                nc.tensor.matmul(nv, fkscol[h][64:128, 0:2],
                                 QST[h][64:128, cs], start=False, stop=True)
                nc.scalar.activation(row2(NVt, NVb, h)[:, cs], nv,
                                     AF.Identity, bias=vwcol_sb[:, 0:1])

        # ======== stage G: k-summaries (no N x N scores needed — the
        # bilinear form re-associates: out = (QST^T KST) fv = QST^T (KST fv))
        for h in range(HPG):
            hs = slice(h * 64, (h + 1) * 64)
            graw = PSS.tile([64, DIM_HEAD], f32, tag="pss")
            gnrm = PSS.tile([64, DIM_HEAD], f32, tag="pss")
            for mt in range(NT):
                nc.tensor.matmul(graw, fkm[mt][:, hs], fv_sb[mt][:, hs],
                                 start=(mt == 0), stop=(mt == NT - 1))
            for mt in range(NT):
                nc.tensor.matmul(gnrm, fknm[mt][:, hs], fv_sb[mt][:, hs],
                                 start=(mt == 0), stop=(mt == NT - 1))
            nc.vector.tensor_copy(G_sb[h][0:64, :], graw)
            nc.scalar.activation(G_sb[h][64:128, :], gnrm, AF.Copy)
            # WAB rows: wA = colsum(G_raw), wB = colsum(fv) — one psum pair
            wab = PSU.tile([2, DIM_HEAD], f32, tag="big")
            nc.tensor.matmul(wab, oc2_sb[0:64, 0:2], G_sb[h][0:64, :],
                             start=True, stop=False)
            for mt in range(NT):
                nc.tensor.matmul(wab, oc2_sb[:, 1:3], fv_sb[mt][:, hs],
                                 start=False, stop=(mt == NT - 1))
            nc.vector.tensor_copy(row2(WABt, WABb, h), wab)

        # ======== stage D: tiny out-stage + E ========
        for ncx in range(NC):
            cs = slice(ncx * 512, (ncx + 1) * 512)
            for h in range(HPG):
                hp, ds = h // 2, (h % 2) * 64
                po = PSU.tile([64, 512], f32, tag="big")
                nc.tensor.matmul(po, G_sb[h], QST[h][:, cs],
                                 start=True, stop=False)
                wabs = (WABt[32 * h:32 * h + 2, :] if h < 3 else WABb[0:2, :])
                nc.tensor.matmul(po, wabs, row2(NVt, NVb, h)[:, cs],
                                 start=False, stop=True)
                nc.scalar.activation(oT2[hp][ds:ds + 64, cs], po, AF.Copy)
            for nt in range(ncx * 4, ncx * 4 + 4):
                pf = PSU.tile([128, 512], f32, tag="big")
                for j in range(2):
                    nc.tensor.matmul(
                        pf, oT2[j][:, nt * 128:(nt + 1) * 128], wo_sb[j],
                        start=(j == 0), stop=(j == 1))
                ob = OSB.tile([128, 512], bf16, tag="ob")
                if nt % 2 == 0:
                    nc.scalar.activation(ob, pf, AF.Copy)
                else:
                    nc.vector.tensor_copy(ob, pf)
                nc.sync.dma_start(out=out[nt * 128:(nt + 1) * 128, :], in_=ob)

    _lp.__exit__(None, None, None)
    nc.compile()
    return nc


def _prep(q, k, v, ln_g, ln_b, W_in, W_out, b_out, cov_w_raw, var_w_raw):
    import ml_dtypes
    bf = ml_dtypes.bfloat16

    q = np.asarray(q, np.float32)
    k = np.asarray(k, np.float32)
    v = np.asarray(v, np.float32)
    ln_g = np.asarray(ln_g, np.float32)
    ln_b = np.asarray(ln_b, np.float32)
    W_in = np.asarray(W_in, np.float32)
    W_out = np.asarray(W_out, np.float32)

    cov_w = float(1.0 / (1.0 + np.exp(-np.float64(cov_w_raw))))
    var_w = float(1.0 / (1.0 + np.exp(-np.float64(var_w_raw))))
    cos_w = 1.0 - cov_w - var_w

    nc = _build_nc(cos_w, cov_w, var_w)

    W_f = (ln_g[:, None] * W_in).astype(np.float32)
    bW = (ln_b @ W_in).astype(np.float32)
    ident = np.eye(128, dtype=np.float32)
    sel2 = np.zeros((128, 2), np.float32)
    sel2[:64, 0] = 1.0
    sel2[64:, 1] = 1.0
    ocol = np.ones((128, 1), np.float32)
    oc2 = np.zeros((128, 3), np.float32)
    oc2[:, 0] = 1.0
    oc2[:, 2] = 1.0
    qcsel = np.zeros((64, 2), np.float32)
    qcsel[:, 0] = -1.0 / DIM_HEAD
    vwcol = np.zeros((2, 1), np.float32)
    vwcol[1, 0] = var_w
    e1v = np.zeros((64, 128), np.float32)
    e1v[0, :] = 1.0
    e2t = np.zeros((66, 128), np.float32)
    for h in range(3):
        e2t[32 * h, :64] = cos_w
        e2t[32 * h + 1, 64:] = 1.0
    e2b = np.zeros((2, 128), np.float32)
    e2b[0, :64] = cos_w
    e2b[1, 64:] = 1.0

    in_maps = []
    for core in range(8):
        b, g = core // HG, core % HG
        bWg = bW[g * IG:(g + 1) * IG]
        in_maps.append({
            "xq": np.ascontiguousarray(q[b]),
            "xk": np.ascontiguousarray(k[b]),
            "xv": np.ascontiguousarray(v[b]),
            "wf": np.ascontiguousarray(
                W_f[:, g * IG:(g + 1) * IG]).astype(bf),
            "wo": np.ascontiguousarray(
                W_out[g * IG:(g + 1) * IG, :]).astype(bf),
            "bwq": np.ascontiguousarray(
                bWg.reshape(HPG, 64).T * (cov_w / DIM_HEAD)).astype(
                    np.float32),
            "bwk": np.ascontiguousarray(
                bWg.reshape(HPG, 64).T).astype(np.float32),
            "bwv": np.concatenate(
                [bWg[None, :], np.zeros((63, IG), np.float32)],
                axis=0).astype(bf),
            "e1v": e1v.astype(bf),
            "ident": ident.astype(bf),
            "sel2": sel2.astype(bf),
            "ocol": ocol.astype(bf),
            "oc2": oc2.astype(bf),
            "qcsel": qcsel.astype(bf),
            "vwcol": vwcol,
            "e2t": e2t.astype(bf),
            "e2b": e2b.astype(bf),
        })
    return nc, in_maps


def kernel(q, k, v, ln_g, ln_b, W_in, W_out, b_out, cov_w_raw, var_w_raw):
    from concourse.bass_utils import run_bass_kernel_spmd

    b_out = np.asarray(b_out, np.float32)
    nc, in_maps = _prep(q, k, v, ln_g, ln_b, W_in, W_out, b_out,
                        cov_w_raw, var_w_raw)
    res = run_bass_kernel_spmd(nc, in_maps, list(range(8)))
    parts = [np.asarray(res.results[c]["out"], np.float32) for c in range(8)]
    out = np.stack([parts[2 * b] + parts[2 * b + 1] + b_out
                    for b in range(B)])
    return out.astype(np.float32)


# revision 11
# speedup vs baseline: 2.9551x; 1.0304x over previous
"""Trainium2 Bass kernel for nn_Attention_30562987278646 — v12.

Sharding: 8 cores = 4 batches x 2 head-groups (4 heads each).

Per core, bf16 data path (2e-2 tolerance):
 A: LN fused into one Pool tensor_scalar (scale+shift) -> bf16 transpose.
 B: projections; W_in bias folded into PSUM->SBUF copies (Act bias col).
    KST[h] = [fTk; fkn], QST[h] = [fqc; fqn] stacked per head.
 C: squares -> sel2 matmul -> norm pairs; fused sqrt/recip; broadcast
    matmul + Pool muls build normalized bottom halves.
    Both rank-1 score terms (mean correction, variance row) are moved to
    the out-stage: NV psum = 3 accumulating matmuls -> rows {nmq, vr};
    wAB[h] = {sum_m Skcol*fv, sum_m fv} via mkcol/ones K=128 matmuls.
 D: score = single K=128 matmul; out-stage accumulates fv^T @ st plus one
    K=2 WAB x NV matmul. Stage E interleaved per n-chunk.

All DMA issue rides the otherwise-idle SP queue (xk loads on Act's HWDGE
to overlap the initial load).
"""

import sys
import numpy as np

for _p in ("/opt/trn_rl_repo", "/root/.axon_site/_ro/trn_rl_repo"):
    if _p not in sys.path:
        sys.path.append(_p)

HEADS = 8
DIM_HEAD = 64
LN_EPS = 1e-5
B, N, DIM = 4, 1024, 512
HG = 2
HPG = HEADS // HG           # heads per group = 4
IG = HPG * DIM_HEAD         # inner dim per group = 256
NT = N // 128               # 8 n-tiles
NC = N // 512               # 2 n-chunks
CC = DIM // 128             # 4 c-chunks


def _build_nc(cos_w: float, cov_w: float, var_w: float):
    import concourse.bass as bass
    import concourse.bacc as bacc
    import concourse.tile as tile
    from concourse import mybir

    f32 = mybir.dt.float32
    f32r = mybir.dt.float32r
    bf16 = mybir.dt.bfloat16
    AF = mybir.ActivationFunctionType
    AX = mybir.AxisListType
    ALU = mybir.AluOpType

    def r(ap):
        return ap.bitcast(f32r)

    nc = bacc.Bacc(target_bir_lowering=False, debug=False)
    _lp = nc.allow_low_precision(reason="2e-2 tolerance; bf16 path validated")
    _lp.__enter__()

    xin_d = {
        "xq": nc.declare_dram_parameter("xq", [N, DIM], f32, isOutput=False),
        "xk": nc.declare_dram_parameter("xk", [N, DIM], f32, isOutput=False),
        "xv": nc.declare_dram_parameter("xv", [N, DIM], f32, isOutput=False),
    }
    wf = nc.declare_dram_parameter("wf", [DIM, IG], bf16, isOutput=False)
    wo = nc.declare_dram_parameter("wo", [IG, DIM], bf16, isOutput=False)
    bwq = nc.declare_dram_parameter("bwq", [64, HPG], f32, isOutput=False)
    bwk = nc.declare_dram_parameter("bwk", [64, HPG], f32, isOutput=False)
    bwv = nc.declare_dram_parameter("bwv", [64, IG], bf16, isOutput=False)
    e1v = nc.declare_dram_parameter("e1v", [64, 128], bf16, isOutput=False)
    ident = nc.declare_dram_parameter("ident", [128, 128], bf16,
                                      isOutput=False)
    sel2 = nc.declare_dram_parameter("sel2", [128, 2], bf16, isOutput=False)
    ocol = nc.declare_dram_parameter("ocol", [128, 1], bf16, isOutput=False)
    oc2 = nc.declare_dram_parameter("oc2", [128, 3], bf16, isOutput=False)
    qcsel = nc.declare_dram_parameter("qcsel", [64, 2], bf16, isOutput=False)
    vwcol = nc.declare_dram_parameter("vwcol", [2, 1], f32, isOutput=False)
    e2t = nc.declare_dram_parameter("e2t", [66, 128], bf16, isOutput=False)
    e2b = nc.declare_dram_parameter("e2b", [2, 128], bf16, isOutput=False)
    out = nc.declare_dram_parameter("out", [N, DIM], bf16, isOutput=True)

    with tile.TileContext(nc) as tc, \
         tc.tile_pool(name="persist", bufs=1) as P, \
         tc.tile_pool(name="stt", bufs=10) as STP, \
         tc.tile_pool(name="small", bufs=6) as SM, \
         tc.tile_pool(name="osb", bufs=4) as OSB, \
         tc.tile_pool(name="psu", bufs=3, space="PSUM") as PSU, \
         tc.tile_pool(name="pss", bufs=5, space="PSUM") as PSS:

        # ---- weights / constants: SP queue, load order = first use ----
        id_sb = P.tile([128, 128], bf16, tag="id", name="id_sb")
        xts = {t: [] for t in ("xq", "xk", "xv")}
        for t in ("xq", "xk", "xv"):
            for nt in range(NT):
                xts[t].append(P.tile([128, DIM], f32, tag=f"{t}_in{nt}",
                                     name=f"{t}_in{nt}"))

        def load_x(t, lo, hi):
            for nt in range(lo, hi):
                nc.sync.dma_start(
                    out=xts[t][nt],
                    in_=xin_d[t][nt * 128:(nt + 1) * 128, :])

        wf_sb = [P.tile([128, IG], bf16, tag=f"wf{c}", name=f"wf{c}")
                 for c in range(CC)]
        bwq_sb = P.tile([64, HPG], f32, tag="bwq", name="bwq_sb")
        bwk_sb = P.tile([64, HPG], f32, tag="bwk", name="bwk_sb")
        bwv_sb = P.tile([64, IG], bf16, tag="bwv", name="bwv_sb")
        e1v_sb = P.tile([64, 128], bf16, tag="e1v", name="e1v_sb")
        # k loads ride the Pool (SWDGE) queue in parallel with SP's q loads
        for nt in range(NT):
            nc.gpsimd.dma_start(
                out=xts["xk"][nt],
                in_=xin_d["xk"][nt * 128:(nt + 1) * 128, :])
        load_x("xq", 0, 1)
        nc.sync.dma_start(out=id_sb, in_=ident[:, :])
        load_x("xq", 1, 6)
        for c in range(CC):
            nc.sync.dma_start(out=wf_sb[c], in_=wf[c * 128:(c + 1) * 128, :])
        nc.sync.dma_start(out=bwq_sb, in_=bwq[:, :])
        nc.sync.dma_start(out=bwk_sb, in_=bwk[:, :])
        load_x("xq", 6, 8)
        load_x("xv", 0, 8)
        nc.sync.dma_start(out=bwv_sb, in_=bwv[:, :])
        nc.sync.dma_start(out=e1v_sb, in_=e1v[:, :])
        sel2_sb = P.tile([128, 2], bf16, tag="sel2", name="sel2_sb")
        nc.sync.dma_start(out=sel2_sb, in_=sel2[:, :])
        ocol_sb = P.tile([128, 1], bf16, tag="ocol", name="ocol_sb")
        nc.sync.dma_start(out=ocol_sb, in_=ocol[:, :])
        oc2_sb = P.tile([128, 3], bf16, tag="oc2", name="oc2_sb")
        nc.sync.dma_start(out=oc2_sb, in_=oc2[:, :])
        qcsel_sb = P.tile([64, 2], bf16, tag="qcsel", name="qcsel_sb")
        nc.sync.dma_start(out=qcsel_sb, in_=qcsel[:, :])
        vwcol_sb = P.tile([2, 1], f32, tag="vwcol", name="vwcol_sb")
        nc.sync.dma_start(out=vwcol_sb, in_=vwcol[:, :])
        e2t_sb = P.tile([66, 128], bf16, tag="e2t", name="e2t_sb")
        nc.sync.dma_start(out=e2t_sb, in_=e2t[:, :])
        e2b_sb = P.tile([2, 128], bf16, tag="e2b", name="e2b_sb")
        nc.sync.dma_start(out=e2b_sb, in_=e2b[:, :])
        wo_sb = [P.tile([128, DIM], bf16, tag=f"wo{j}", name=f"wo{j}")
                 for j in range(2)]
        for j in range(2):
            nc.sync.dma_start(out=wo_sb[j], in_=wo[j * 128:(j + 1) * 128, :])

        eps_sb = P.tile([128, 1], f32, tag="eps", name="eps_sb")
        nc.gpsimd.memset(eps_sb, LN_EPS)
        # prewarm the Sqrt activation table off the critical path
        warm = P.tile([1, 1], f32, tag="warm", name="warm")
        nc.scalar.activation(warm, eps_sb[0:1, 0:1], AF.Sqrt)

        # ---- persistent activations ----
        QST = [P.tile([128, N], bf16, tag=f"QST{h}", name=f"QST{h}")
               for h in range(HPG)]
        fv_sb = [P.tile([128, IG], bf16, tag=f"fv{mt}", name=f"fv{mt}")
                 for mt in range(NT)]
        SQ = [P.tile([128, N], bf16, tag=f"SQ{h}", name=f"SQ{h}")
              for h in range(HPG)]
        # row pairs at partition 32h (h<3) / sibling tiles (h=3)
        NVt = P.tile([66, N], bf16, tag="NVt", name="NVt")
        NVb = P.tile([2, N], bf16, tag="NVb", name="NVb")
        WABt = P.tile([66, DIM_HEAD], bf16, tag="WABt", name="WABt")
        WABb = P.tile([2, DIM_HEAD], bf16, tag="WABb", name="WABb")
        NRt = P.tile([66, N], bf16, tag="NRt", name="NRt")
        NRb = P.tile([2, N], bf16, tag="NRb", name="NRb")
        fkscol = [P.tile([128, 2], bf16, tag=f"fks{h}", name=f"fks{h}")
                  for h in range(HPG)]
        pbq0 = P.tile([64, N], bf16, tag="pbq0", name="pbq0")
        pbk0 = P.tile([64, N], bf16, tag="pbk0", name="pbk0")
        fkm = [P.tile([128, IG], bf16, tag=f"fkm{mt}", name=f"fkm{mt}")
               for mt in range(NT)]
        fknm = [P.tile([128, IG], bf16, tag=f"fknm{mt}", name=f"fknm{mt}")
                for mt in range(NT)]
        rkncol = [P.tile([128, NT], f32, tag=f"rkc{h}", name=f"rkc{h}")
                  for h in range(HPG)]
        G_sb = [P.tile([128, DIM_HEAD], bf16, tag=f"G{h}", name=f"G{h}")
                for h in range(HPG)]
        oT2 = [P.tile([128, N], bf16, tag=f"oT2{j}", name=f"oT2{j}")
               for j in range(2)]

        def row2(tm, tb, h):
            return tm[32 * h:32 * h + 2, :] if h < 3 else tb[0:2, :]

        # ======== stages A+B ========
        with tc.tile_pool(name="xtp", bufs=1) as XT, \
             tc.tile_pool(name="xin", bufs=3) as XIN:
            xT4 = {t: [XT.tile([128, DIM], bf16, tag=f"xT4{t}{nt}",
                               name=f"xT4{t}{nt}") for nt in range(NT)]
                   for t in ("xq", "xk", "xv")}
            xT4["xk2"] = xT4["xk"]

            def stage_a_nt(t, nt):
                    xt = xts[t][nt]
                    stats = SM.tile([128, nc.vector.BN_STATS_DIM], f32,
                                    tag="bns")
                    nc.vector.bn_stats(out=stats, in_=xt)
                    mv = SM.tile([128, nc.vector.BN_AGGR_DIM], f32, tag="bna")
                    nc.vector.bn_aggr(out=mv, in_=stats)
                    std = SM.tile([128, 1], f32, tag="std")
                    nc.scalar.activation(std, mv[:, 1:2], AF.Sqrt,
                                         bias=eps_sb)
                    rin = SM.tile([128, 1], f32, tag="rin")
                    nc.vector.reciprocal(rin, std)
                    nmr = SM.tile([128, 1], f32, tag="nmr")
                    nc.vector.scalar_tensor_tensor(
                        nmr, mv[:, 0:1], -1.0, rin, ALU.mult, ALU.mult)
                    xln = XIN.tile([128, DIM], bf16, tag="xln")
                    nc.gpsimd.tensor_scalar(
                        xln, xt, rin, nmr, ALU.mult, ALU.add)
                    pt = PSS.tile([128, DIM], bf16, tag="pss")
                    for c in range(CC):
                        nc.tensor.transpose(
                            pt[:, c * 128:(c + 1) * 128],
                            xln[:, c * 128:(c + 1) * 128], id_sb)
                    if nt % 2 == 0:
                        nc.vector.tensor_copy(xT4[t][nt], pt)
                    else:
                        nc.scalar.activation(xT4[t][nt], pt, AF.Copy)

            def stage_b_qk(t):
                dst, bcol, scl = (QST, bwq_sb, cov_w / DIM_HEAD)
                for nt in range(NT):
                    ts = slice(nt * 128, (nt + 1) * 128)
                    for hp in range(2):
                        pf = PSU.tile([128, 128], f32, tag="big")
                        for c in range(CC):
                            nc.tensor.matmul(
                                pf,
                                wf_sb[c][:, hp * 128:(hp + 1) * 128],
                                xT4[t][nt][:, c * 128:(c + 1) * 128],
                                start=(c == 0), stop=(c == CC - 1))
                        for hj in range(2):
                            h = 2 * hp + hj
                            src = pf[hj * 64:hj * 64 + 64, 0:128]
                            if (nt + hp) % 2 == 0:
                                nc.scalar.activation(
                                    dst[h][0:64, ts], src, AF.Identity,
                                    bias=bcol[:, h:h + 1], scale=scl)
                            else:
                                nc.vector.tensor_scalar(
                                    dst[h][0:64, ts], src, scl,
                                    bcol[:, h:h + 1], ALU.mult, ALU.add)

            def stage_c_sq(tiles, half):
                # squares into SQ halves (top: fqc^2, bottom: fTk^2)
                for h in range(HPG):
                    nc.gpsimd.tensor_mul(SQ[h][half, :], tiles[h][0:64, :],
                                         tiles[h][0:64, :])

            for nt in range(NT):
                stage_a_nt("xq", nt)
                stage_a_nt("xk", nt)
            stage_b_qk("xq")
            stage_c_sq(QST, slice(0, 64))
            # token-major k projections (B_k d-major path deleted)
            for mt in range(NT):
                pf = PSU.tile([128, IG], f32, tag="big")
                for c in range(CC):
                    nc.tensor.matmul(
                        pf, xT4["xk"][mt][:, c * 128:(c + 1) * 128],
                        wf_sb[c], start=(c == 0), stop=False)
                nc.tensor.matmul(pf, e1v_sb, bwv_sb, start=False, stop=True)
                nc.scalar.activation(fkm[mt], pf, AF.Copy)
            # per-token 1/kn columns: square-accum + batched sqrt/recip
            for h in range(HPG):
                hsl = slice(h * 64, (h + 1) * 64)
                for mt in range(NT):
                    scr = SM.tile([128, DIM_HEAD], bf16, tag="scr")
                    nc.vector.scalar_tensor_tensor(
                        scr, fkm[mt][:, hsl], 1.0, fkm[mt][:, hsl],
                        ALU.bypass, ALU.mult,
                        accum_out=rkncol[h][:, mt:mt + 1])
                nc.scalar.activation(rkncol[h], rkncol[h], AF.Sqrt)
                nc.vector.reciprocal(rkncol[h], rkncol[h])
            # norm pairs -> NRt rows; fused sqrt + recip
            nc.gpsimd.memset(NRt, 1.0)
            nc.gpsimd.memset(NRb, 1.0)
            for h in range(HPG):
                nc.gpsimd.memset(fkscol[h], 0.0)
            for h in range(HPG):
                for ncx in range(NC):
                    cs = slice(ncx * 512, (ncx + 1) * 512)
                    nr2 = PSU.tile([1, 512], f32, tag="big")
                    nc.tensor.matmul(nr2, sel2_sb[0:64, 0:1],
                                     SQ[h][0:64, cs], start=True, stop=True)
                    nc.scalar.activation(row2(NRt, NRb, h)[0:1, cs], nr2,
                                         AF.Copy)
            nc.scalar.activation(NRt, NRt, AF.Sqrt)
            nc.vector.reciprocal(NRt, NRt)
            nc.scalar.activation(NRb, NRb, AF.Sqrt)
            nc.vector.reciprocal(NRb, NRb)

            for nt in range(NT):
                stage_a_nt("xv", nt)
            for mt in range(NT):
                pf = PSU.tile([128, IG], f32, tag="big")
                for c in range(CC):
                    nc.tensor.matmul(
                        pf, xT4["xv"][mt][:, c * 128:(c + 1) * 128],
                        wf_sb[c], start=(c == 0), stop=False)
                nc.tensor.matmul(pf, e1v_sb, bwv_sb, start=False, stop=True)
                nc.scalar.activation(fv_sb[mt], pf, AF.Copy)
            for h in range(HPG):
                hsl = slice(h * 64, (h + 1) * 64)
                for mt in range(NT):
                    nc.gpsimd.tensor_scalar(
                        fknm[mt][:, hsl], fkm[mt][:, hsl],
                        rkncol[h][:, mt:mt + 1], 0.0, ALU.mult, ALU.add)

        # ======== stage C tail: normalized halves, NV rows ========
        vr_scale = -(var_w / (N * cos_w))
        for h in range(HPG):
            e2s = (e2t_sb[32 * h:32 * h + 2, :] if h < 3 else e2b_sb[0:2, :])
            for ncx in range(NC):
                cs = slice(ncx * 512, (ncx + 1) * 512)
                pb = PSS.tile([128, 512], f32, tag="pss")
                nc.tensor.matmul(pb, e2s, row2(NRt, NRb, h)[:, cs],
                                 start=True, stop=True)
                nc.vector.tensor_copy(pbq0[:, cs], pb[0:64, 0:512])
                nc.gpsimd.tensor_mul(QST[h][64:128, cs],
                                     QST[h][0:64, cs], pbq0[:, cs])
            # fks from fknm: row accumulate -> transpose -> scaled column
            hsl = slice(h * 64, (h + 1) * 64)
            fkr = PSS.tile([1, DIM_HEAD], f32, tag="pss")
            for mt in range(NT):
                nc.tensor.matmul(fkr, ocol_sb[:, 0:1], fknm[mt][:, hsl],
                                 start=(mt == 0), stop=(mt == NT - 1))
            fkrs = SM.tile([1, DIM_HEAD], bf16, tag="fkrs")
            nc.vector.tensor_copy(fkrs, fkr)
            fkc = PSS.tile([DIM_HEAD, 1], bf16, tag="pss")
            nc.tensor.transpose(fkc, fkrs, id_sb[0:1, 0:1])
            nc.scalar.activation(fkscol[h][64:128, 1:2], fkc, AF.Identity,
                                 scale=vr_scale)
            for ncx in range(NC):
                cs = slice(ncx * 512, (ncx + 1) * 512)
                nv = PSU.tile([2, 512], f32, tag="big")
                nc.tensor.matmul(nv, qcsel_sb, QST[h][0:64, cs],
                                 start=True, stop=False)
                nc.tensor.matmul(nv, fkscol[h][64:128, 0:2],
                                 QST[h][64:128, cs], start=False, stop=True)
                nc.scalar.activation(row2(NVt, NVb, h)[:, cs], nv,
                                     AF.Identity, bias=vwcol_sb[:, 0:1])

        # ======== stage G: k-summaries (no N x N scores needed — the
        # bilinear form re-associates: out = (QST^T KST) fv = QST^T (KST fv))
        for h in range(HPG):
            hs = slice(h * 64, (h + 1) * 64)
            graw = PSS.tile([64, DIM_HEAD], f32, tag="pss")
            gnrm = PSS.tile([64, DIM_HEAD], f32, tag="pss")
            for mt in range(NT):
                nc.tensor.matmul(graw, fkm[mt][:, hs], fv_sb[mt][:, hs],
                                 start=(mt == 0), stop=(mt == NT - 1))
            for mt in range(NT):
                nc.tensor.matmul(gnrm, fknm[mt][:, hs], fv_sb[mt][:, hs],
                                 start=(mt == 0), stop=(mt == NT - 1))
            nc.vector.tensor_copy(G_sb[h][0:64, :], graw)
            nc.scalar.activation(G_sb[h][64:128, :], gnrm, AF.Copy)
            # WAB rows: wA = colsum(G_raw), wB = colsum(fv) — one psum pair
            wab = PSU.tile([2, DIM_HEAD], f32, tag="big")
            nc.tensor.matmul(wab, oc2_sb[0:64, 0:2], G_sb[h][0:64, :],
                             start=True, stop=False)
            for mt in range(NT):
                nc.tensor.matmul(wab, oc2_sb[:, 1:3], fv_sb[mt][:, hs],
                                 start=False, stop=(mt == NT - 1))
            nc.vector.tensor_copy(row2(WABt, WABb, h), wab)

        # ======== stage D: tiny out-stage + E ========
        for ncx in range(NC):
            cs = slice(ncx * 512, (ncx + 1) * 512)
            for h in range(HPG):
                hp, ds = h // 2, (h % 2) * 64
                po = PSU.tile([64, 512], f32, tag="big")
                nc.tensor.matmul(po, G_sb[h], QST[h][:, cs],
                                 start=True, stop=False)
                wabs = (WABt[32 * h:32 * h + 2, :] if h < 3 else WABb[0:2, :])
                nc.tensor.matmul(po, wabs, row2(NVt, NVb, h)[:, cs],
                                 start=False, stop=True)
                nc.scalar.activation(oT2[hp][ds:ds + 64, cs], po, AF.Copy)
            for nt in range(ncx * 4, ncx * 4 + 4):
                pf = PSU.tile([128, 512], f32, tag="big")
                for j in range(2):
                    nc.tensor.matmul(
                        pf, oT2[j][:, nt * 128:(nt + 1) * 128], wo_sb[j],
                        start=(j == 0), stop=(j == 1))
                ob = OSB.tile([128, 512], bf16, tag="ob")
                if nt % 2 == 0:
                    nc.scalar.activation(ob, pf, AF.Copy)
                else:
                    nc.vector.tensor_copy(ob, pf)
                nc.sync.dma_start(out=out[nt * 128:(nt + 1) * 128, :], in_=ob)

    _lp.__exit__(None, None, None)
    nc.compile()
    return nc


def _prep(q, k, v, ln_g, ln_b, W_in, W_out, b_out, cov_w_raw, var_w_raw):
    import ml_dtypes
    bf = ml_dtypes.bfloat16

    q = np.asarray(q, np.float32)
    k = np.asarray(k, np.float32)
    v = np.asarray(v, np.float32)
    ln_g = np.asarray(ln_g, np.float32)
    ln_b = np.asarray(ln_b, np.float32)
    W_in = np.asarray(W_in, np.float32)
    W_out = np.asarray(W_out, np.float32)

    cov_w = float(1.0 / (1.0 + np.exp(-np.float64(cov_w_raw))))
    var_w = float(1.0 / (1.0 + np.exp(-np.float64(var_w_raw))))
    cos_w = 1.0 - cov_w - var_w

    nc = _build_nc(cos_w, cov_w, var_w)

    W_f = (ln_g[:, None] * W_in).astype(np.float32)
    bW = (ln_b @ W_in).astype(np.float32)
    ident = np.eye(128, dtype=np.float32)
    sel2 = np.zeros((128, 2), np.float32)
    sel2[:64, 0] = 1.0
    sel2[64:, 1] = 1.0
    ocol = np.ones((128, 1), np.float32)
    oc2 = np.zeros((128, 3), np.float32)
    oc2[:, 0] = 1.0
    oc2[:, 2] = 1.0
    qcsel = np.zeros((64, 2), np.float32)
    qcsel[:, 0] = -1.0 / DIM_HEAD
    vwcol = np.zeros((2, 1), np.float32)
    vwcol[1, 0] = var_w
    e1v = np.zeros((64, 128), np.float32)
    e1v[0, :] = 1.0
    e2t = np.zeros((66, 128), np.float32)
    for h in range(3):
        e2t[32 * h, :64] = cos_w
        e2t[32 * h + 1, 64:] = 1.0
    e2b = np.zeros((2, 128), np.float32)
    e2b[0, :64] = cos_w
    e2b[1, 64:] = 1.0

    in_maps = []
    for core in range(8):
        b, g = core // HG, core % HG
        bWg = bW[g * IG:(g + 1) * IG]
        in_maps.append({
            "xq": np.ascontiguousarray(q[b]),
            "xk": np.ascontiguousarray(k[b]),
            "xv": np.ascontiguousarray(v[b]),
            "wf": np.ascontiguousarray(
                W_f[:, g * IG:(g + 1) * IG]).astype(bf),
            "wo": np.ascontiguousarray(
                W_out[g * IG:(g + 1) * IG, :]).astype(bf),
            "bwq": np.ascontiguousarray(
                bWg.reshape(HPG, 64).T * (cov_w / DIM_HEAD)).astype(
                    np.float32),
            "bwk": np.ascontiguousarray(
                bWg.reshape(HPG, 64).T).astype(np.float32),
            "bwv": np.concatenate(
                [bWg[None, :], np.zeros((63, IG), np.float32)],
                axis=0).astype(bf),
            "e1v": e1v.astype(bf),
            "ident": ident.astype(bf),
            "sel2": sel2.astype(bf),
            "ocol": ocol.astype(bf),
            "oc2": oc2.astype(bf),
            "qcsel": qcsel.astype(bf),
            "vwcol": vwcol,
            "e2t": e2t.astype(bf),
            "e2b": e2b.astype(bf),
        })
    return nc, in_maps


def kernel(q, k, v, ln_g, ln_b, W_in, W_out, b_out, cov_w_raw, var_w_raw):
    from concourse.bass_utils import run_bass_kernel_spmd

    b_out = np.asarray(b_out, np.float32)
    nc, in_maps = _prep(q, k, v, ln_g, ln_b, W_in, W_out, b_out,
                        cov_w_raw, var_w_raw)
    res = run_bass_kernel_spmd(nc, in_maps, list(range(8)))
    parts = [np.asarray(res.results[c]["out"], np.float32) for c in range(8)]
    out = np.stack([parts[2 * b] + parts[2 * b + 1] + b_out
                    for b in range(B)])
    return out.astype(np.float32)


# revision 12
# speedup vs baseline: 2.9886x; 1.0114x over previous
"""Trainium2 Bass kernel for nn_Attention_30562987278646 — v12.

Sharding: 8 cores = 4 batches x 2 head-groups (4 heads each).

Per core, bf16 data path (2e-2 tolerance):
 A: LN fused into one Pool tensor_scalar (scale+shift) -> bf16 transpose.
 B: projections; W_in bias folded into PSUM->SBUF copies (Act bias col).
    KST[h] = [fTk; fkn], QST[h] = [fqc; fqn] stacked per head.
 C: squares -> sel2 matmul -> norm pairs; fused sqrt/recip; broadcast
    matmul + Pool muls build normalized bottom halves.
    Both rank-1 score terms (mean correction, variance row) are moved to
    the out-stage: NV psum = 3 accumulating matmuls -> rows {nmq, vr};
    wAB[h] = {sum_m Skcol*fv, sum_m fv} via mkcol/ones K=128 matmuls.
 D: score = single K=128 matmul; out-stage accumulates fv^T @ st plus one
    K=2 WAB x NV matmul. Stage E interleaved per n-chunk.

All DMA issue rides the otherwise-idle SP queue (xk loads on Act's HWDGE
to overlap the initial load).
"""

import sys
import numpy as np

for _p in ("/opt/trn_rl_repo", "/root/.axon_site/_ro/trn_rl_repo"):
    if _p not in sys.path:
        sys.path.append(_p)

HEADS = 8
DIM_HEAD = 64
LN_EPS = 1e-5
B, N, DIM = 4, 1024, 512
HG = 2
HPG = HEADS // HG           # heads per group = 4
IG = HPG * DIM_HEAD         # inner dim per group = 256
NT = N // 128               # 8 n-tiles
NC = N // 512               # 2 n-chunks
CC = DIM // 128             # 4 c-chunks


def _build_nc(cos_w: float, cov_w: float, var_w: float):
    import concourse.bass as bass
    import concourse.bacc as bacc
    import concourse.tile as tile
    from concourse import mybir

    f32 = mybir.dt.float32
    f32r = mybir.dt.float32r
    bf16 = mybir.dt.bfloat16
    AF = mybir.ActivationFunctionType
    AX = mybir.AxisListType
    ALU = mybir.AluOpType

    def r(ap):
        return ap.bitcast(f32r)

    nc = bacc.Bacc(target_bir_lowering=False, debug=False)
    _lp = nc.allow_low_precision(reason="2e-2 tolerance; bf16 path validated")
    _lp.__enter__()

    xin_d = {
        "xq": nc.declare_dram_parameter("xq", [N, DIM], f32, isOutput=False),
        "xk": nc.declare_dram_parameter("xk", [N, DIM], f32, isOutput=False),
        "xv": nc.declare_dram_parameter("xv", [N, DIM], f32, isOutput=False),
    }
    wf = nc.declare_dram_parameter("wf", [DIM, IG], bf16, isOutput=False)
    wo = nc.declare_dram_parameter("wo", [IG, DIM], bf16, isOutput=False)
    bwq = nc.declare_dram_parameter("bwq", [64, HPG], f32, isOutput=False)
    bwk = nc.declare_dram_parameter("bwk", [64, HPG], f32, isOutput=False)
    bwv = nc.declare_dram_parameter("bwv", [64, IG], bf16, isOutput=False)
    e1v = nc.declare_dram_parameter("e1v", [64, 128], bf16, isOutput=False)
    ident = nc.declare_dram_parameter("ident", [128, 128], bf16,
                                      isOutput=False)
    sel2 = nc.declare_dram_parameter("sel2", [128, 2], bf16, isOutput=False)
    ocol = nc.declare_dram_parameter("ocol", [128, 1], bf16, isOutput=False)
    oc2 = nc.declare_dram_parameter("oc2", [128, 3], bf16, isOutput=False)
    qcsel = nc.declare_dram_parameter("qcsel", [64, 2], bf16, isOutput=False)
    vwcol = nc.declare_dram_parameter("vwcol", [2, 1], f32, isOutput=False)
    e2t = nc.declare_dram_parameter("e2t", [66, 128], bf16, isOutput=False)
    e2b = nc.declare_dram_parameter("e2b", [2, 128], bf16, isOutput=False)
    out = nc.declare_dram_parameter("out", [N, DIM], bf16, isOutput=True)

    with tile.TileContext(nc) as tc, \
         tc.tile_pool(name="persist", bufs=1) as P, \
         tc.tile_pool(name="stt", bufs=10) as STP, \
         tc.tile_pool(name="small", bufs=6) as SM, \
         tc.tile_pool(name="osb", bufs=4) as OSB, \
         tc.tile_pool(name="psu", bufs=3, space="PSUM") as PSU, \
         tc.tile_pool(name="pss", bufs=5, space="PSUM") as PSS:

        # ---- weights / constants: SP queue, load order = first use ----
        id_sb = P.tile([128, 128], bf16, tag="id", name="id_sb")
        xts = {t: [] for t in ("xq", "xk", "xv")}
        for t in ("xq", "xk", "xv"):
            for nt in range(NT):
                xts[t].append(P.tile([128, DIM], f32, tag=f"{t}_in{nt}",
                                     name=f"{t}_in{nt}"))

        def load_x(t, lo, hi):
            for nt in range(lo, hi):
                nc.sync.dma_start(
                    out=xts[t][nt],
                    in_=xin_d[t][nt * 128:(nt + 1) * 128, :])

        wf_sb = [P.tile([128, IG], bf16, tag=f"wf{c}", name=f"wf{c}")
                 for c in range(CC)]
        bwq_sb = P.tile([64, HPG], f32, tag="bwq", name="bwq_sb")
        bwk_sb = P.tile([64, HPG], f32, tag="bwk", name="bwk_sb")
        bwv_sb = P.tile([64, IG], bf16, tag="bwv", name="bwv_sb")
        e1v_sb = P.tile([64, 128], bf16, tag="e1v", name="e1v_sb")
        # k loads ride the Pool (SWDGE) queue in parallel with SP's q loads
        for nt in range(NT):
            nc.gpsimd.dma_start(
                out=xts["xk"][nt],
                in_=xin_d["xk"][nt * 128:(nt + 1) * 128, :])
        load_x("xq", 0, 1)
        nc.sync.dma_start(out=id_sb, in_=ident[:, :])
        load_x("xq", 1, 6)
        for c in range(CC):
            nc.sync.dma_start(out=wf_sb[c], in_=wf[c * 128:(c + 1) * 128, :])
        nc.sync.dma_start(out=bwq_sb, in_=bwq[:, :])
        nc.sync.dma_start(out=bwk_sb, in_=bwk[:, :])
        load_x("xq", 6, 8)
        load_x("xv", 0, 8)
        nc.sync.dma_start(out=bwv_sb, in_=bwv[:, :])
        nc.sync.dma_start(out=e1v_sb, in_=e1v[:, :])
        sel2_sb = P.tile([128, 2], bf16, tag="sel2", name="sel2_sb")
        nc.sync.dma_start(out=sel2_sb, in_=sel2[:, :])
        ocol_sb = P.tile([128, 1], bf16, tag="ocol", name="ocol_sb")
        nc.sync.dma_start(out=ocol_sb, in_=ocol[:, :])
        oc2_sb = P.tile([128, 3], bf16, tag="oc2", name="oc2_sb")
        nc.sync.dma_start(out=oc2_sb, in_=oc2[:, :])
        qcsel_sb = P.tile([64, 2], bf16, tag="qcsel", name="qcsel_sb")
        nc.sync.dma_start(out=qcsel_sb, in_=qcsel[:, :])
        vwcol_sb = P.tile([2, 1], f32, tag="vwcol", name="vwcol_sb")
        nc.sync.dma_start(out=vwcol_sb, in_=vwcol[:, :])
        e2t_sb = P.tile([66, 128], bf16, tag="e2t", name="e2t_sb")
        nc.sync.dma_start(out=e2t_sb, in_=e2t[:, :])
        e2b_sb = P.tile([2, 128], bf16, tag="e2b", name="e2b_sb")
        nc.sync.dma_start(out=e2b_sb, in_=e2b[:, :])
        wo_sb = [P.tile([128, DIM], bf16, tag=f"wo{j}", name=f"wo{j}")
                 for j in range(2)]
        for j in range(2):
            nc.sync.dma_start(out=wo_sb[j], in_=wo[j * 128:(j + 1) * 128, :])

        eps_sb = P.tile([128, 1], f32, tag="eps", name="eps_sb")
        nc.gpsimd.memset(eps_sb, LN_EPS)
        # prewarm the Sqrt activation table off the critical path
        warm = P.tile([1, 1], f32, tag="warm", name="warm")
        nc.scalar.activation(warm, eps_sb[0:1, 0:1], AF.Sqrt)

        # ---- persistent activations ----
        QST = [P.tile([128, N], bf16, tag=f"QST{h}", name=f"QST{h}")
               for h in range(HPG)]
        fv_sb = [P.tile([128, IG], bf16, tag=f"fv{mt}", name=f"fv{mt}")
                 for mt in range(NT)]
        SQ = [P.tile([128, N], bf16, tag=f"SQ{h}", name=f"SQ{h}")
              for h in range(HPG)]
        # row pairs at partition 32h (h<3) / sibling tiles (h=3)
        NVt = P.tile([66, N], bf16, tag="NVt", name="NVt")
        NVb = P.tile([2, N], bf16, tag="NVb", name="NVb")
        WABt = P.tile([66, DIM_HEAD], bf16, tag="WABt", name="WABt")
        WABb = P.tile([2, DIM_HEAD], bf16, tag="WABb", name="WABb")
        NRt = P.tile([66, N], bf16, tag="NRt", name="NRt")
        NRb = P.tile([2, N], bf16, tag="NRb", name="NRb")
        fkscol = [P.tile([128, 2], bf16, tag=f"fks{h}", name=f"fks{h}")
                  for h in range(HPG)]
        pbq0 = P.tile([64, N], bf16, tag="pbq0", name="pbq0")
        pbk0 = P.tile([64, N], bf16, tag="pbk0", name="pbk0")
        fkm = [P.tile([128, IG], bf16, tag=f"fkm{mt}", name=f"fkm{mt}")
               for mt in range(NT)]
        fknm = [P.tile([128, IG], bf16, tag=f"fknm{mt}", name=f"fknm{mt}")
                for mt in range(NT)]
        rkncol = [P.tile([128, NT], f32, tag=f"rkc{h}", name=f"rkc{h}")
                  for h in range(HPG)]
        G_sb = [P.tile([128, DIM_HEAD], bf16, tag=f"G{h}", name=f"G{h}")
                for h in range(HPG)]
        oT2 = [P.tile([128, N], bf16, tag=f"oT2{j}", name=f"oT2{j}")
               for j in range(2)]

        def row2(tm, tb, h):
            return tm[32 * h:32 * h + 2, :] if h < 3 else tb[0:2, :]

        # ======== stages A+B ========
        with tc.tile_pool(name="xtp", bufs=1) as XT, \
             tc.tile_pool(name="xin", bufs=3) as XIN:
            xT4 = {t: [XT.tile([128, DIM], bf16, tag=f"xT4{t}{nt}",
                               name=f"xT4{t}{nt}") for nt in range(NT)]
                   for t in ("xq", "xk", "xv")}
            xT4["xk2"] = xT4["xk"]

            def stage_a_nt(t, nt):
                    xt = xts[t][nt]
                    stats = SM.tile([128, nc.vector.BN_STATS_DIM], f32,
                                    tag="bns")
                    nc.vector.bn_stats(out=stats, in_=xt)
                    mv = SM.tile([128, nc.vector.BN_AGGR_DIM], f32, tag="bna")
                    nc.vector.bn_aggr(out=mv, in_=stats)
                    std = SM.tile([128, 1], f32, tag="std")
                    nc.scalar.activation(std, mv[:, 1:2], AF.Sqrt,
                                         bias=eps_sb)
                    rin = SM.tile([128, 1], f32, tag="rin")
                    nc.vector.reciprocal(rin, std)
                    nmr = SM.tile([128, 1], f32, tag="nmr")
                    nc.vector.scalar_tensor_tensor(
                        nmr, mv[:, 0:1], -1.0, rin, ALU.mult, ALU.mult)
                    xln = XIN.tile([128, DIM], bf16, tag="xln")
                    nc.gpsimd.tensor_scalar(
                        xln, xt, rin, nmr, ALU.mult, ALU.add)
                    pt = PSS.tile([128, DIM], bf16, tag="pss")
                    for c in range(CC):
                        nc.tensor.transpose(
                            pt[:, c * 128:(c + 1) * 128],
                            xln[:, c * 128:(c + 1) * 128], id_sb)
                    if nt % 2 == 0:
                        nc.vector.tensor_copy(xT4[t][nt], pt)
                    else:
                        nc.scalar.activation(xT4[t][nt], pt, AF.Copy)

            def stage_b_qk(t):
                dst, bcol, scl = (QST, bwq_sb, cov_w / DIM_HEAD)
                for nt in range(NT):
                    ts = slice(nt * 128, (nt + 1) * 128)
                    for hp in range(2):
                        pf = PSU.tile([128, 128], f32, tag="big")
                        for c in range(CC):
                            nc.tensor.matmul(
                                pf,
                                wf_sb[c][:, hp * 128:(hp + 1) * 128],
                                xT4[t][nt][:, c * 128:(c + 1) * 128],
                                start=(c == 0), stop=(c == CC - 1))
                        for hj in range(2):
                            h = 2 * hp + hj
                            src = pf[hj * 64:hj * 64 + 64, 0:128]
                            if (nt + hp) % 2 == 0:
                                nc.scalar.activation(
                                    dst[h][0:64, ts], src, AF.Identity,
                                    bias=bcol[:, h:h + 1], scale=scl)
                            else:
                                nc.vector.tensor_scalar(
                                    dst[h][0:64, ts], src, scl,
                                    bcol[:, h:h + 1], ALU.mult, ALU.add)

            def stage_c_sq(tiles, half):
                # squares into SQ halves (top: fqc^2, bottom: fTk^2)
                for h in range(HPG):
                    nc.gpsimd.tensor_mul(SQ[h][half, :], tiles[h][0:64, :],
                                         tiles[h][0:64, :])

            for nt in range(NT):
                stage_a_nt("xq", nt)
                stage_a_nt("xk", nt)
            stage_b_qk("xq")
            stage_c_sq(QST, slice(0, 64))
            # token-major k projections (B_k d-major path deleted)
            for mt in range(NT):
                pf = PSU.tile([128, IG], f32, tag="big")
                for c in range(CC):
                    nc.tensor.matmul(
                        pf, xT4["xk"][mt][:, c * 128:(c + 1) * 128],
                        wf_sb[c], start=(c == 0), stop=False)
                nc.tensor.matmul(pf, e1v_sb, bwv_sb, start=False, stop=True)
                nc.scalar.activation(fkm[mt], pf, AF.Copy)
            # per-token 1/kn columns: square-accum + batched sqrt/recip
            for h in range(HPG):
                hsl = slice(h * 64, (h + 1) * 64)
                for mt in range(NT):
                    scr = SM.tile([128, DIM_HEAD], bf16, tag="scr")
                    nc.vector.scalar_tensor_tensor(
                        scr, fkm[mt][:, hsl], 1.0, fkm[mt][:, hsl],
                        ALU.bypass, ALU.mult,
                        accum_out=rkncol[h][:, mt:mt + 1])
                nc.scalar.activation(rkncol[h], rkncol[h], AF.Sqrt)
                nc.vector.reciprocal(rkncol[h], rkncol[h])
            # norm pairs -> NRt rows; fused sqrt + recip
            nc.gpsimd.memset(NRt, 1.0)
            nc.gpsimd.memset(NRb, 1.0)
            for h in range(HPG):
                nc.gpsimd.memset(fkscol[h], 0.0)
            for h in range(HPG):
                for ncx in range(NC):
                    cs = slice(ncx * 512, (ncx + 1) * 512)
                    nr2 = PSU.tile([1, 512], f32, tag="big")
                    nc.tensor.matmul(nr2, sel2_sb[0:64, 0:1],
                                     SQ[h][0:64, cs], start=True, stop=True)
                    nc.scalar.activation(row2(NRt, NRb, h)[0:1, cs], nr2,
                                         AF.Copy)
            nc.scalar.activation(NRt, NRt, AF.Sqrt)
            nc.vector.reciprocal(NRt, NRt)
            nc.scalar.activation(NRb, NRb, AF.Sqrt)
            nc.vector.reciprocal(NRb, NRb)

            for nt in range(NT):
                stage_a_nt("xv", nt)
            for mt in range(NT):
                pf = PSU.tile([128, IG], f32, tag="big")
                for c in range(CC):
                    nc.tensor.matmul(
                        pf, xT4["xv"][mt][:, c * 128:(c + 1) * 128],
                        wf_sb[c], start=(c == 0), stop=False)
                nc.tensor.matmul(pf, e1v_sb, bwv_sb, start=False, stop=True)
                nc.scalar.activation(fv_sb[mt], pf, AF.Copy)
            for h in range(HPG):
                hsl = slice(h * 64, (h + 1) * 64)
                for mt in range(NT):
                    nc.gpsimd.tensor_scalar(
                        fknm[mt][:, hsl], fkm[mt][:, hsl],
                        rkncol[h][:, mt:mt + 1], 0.0, ALU.mult, ALU.add)

        # ======== stage C tail: normalized halves, NV rows ========
        vr_scale = -(var_w / (N * cos_w))
        for h in range(HPG):
            e2s = (e2t_sb[32 * h:32 * h + 2, :] if h < 3 else e2b_sb[0:2, :])
            for ncx in range(NC):
                cs = slice(ncx * 512, (ncx + 1) * 512)
                pb = PSS.tile([128, 512], f32, tag="pss")
                nc.tensor.matmul(pb, e2s, row2(NRt, NRb, h)[:, cs],
                                 start=True, stop=True)
                if h % 2 == 0:
                    nc.vector.tensor_copy(pbq0[:, cs], pb[0:64, 0:512])
                else:
                    nc.scalar.activation(pbq0[:, cs], pb[0:64, 0:512],
                                         AF.Copy)
                nc.gpsimd.tensor_mul(QST[h][64:128, cs],
                                     QST[h][0:64, cs], pbq0[:, cs])
            # fks from fknm: row accumulate -> transpose -> scaled column
            hsl = slice(h * 64, (h + 1) * 64)
            fkr = PSS.tile([1, DIM_HEAD], f32, tag="pss")
            for mt in range(NT):
                nc.tensor.matmul(fkr, ocol_sb[:, 0:1], fknm[mt][:, hsl],
                                 start=(mt == 0), stop=(mt == NT - 1))
            fkrs = SM.tile([1, DIM_HEAD], bf16, tag="fkrs")
            nc.vector.tensor_copy(fkrs, fkr)
            fkc = PSS.tile([DIM_HEAD, 1], bf16, tag="pss")
            nc.tensor.transpose(fkc, fkrs, id_sb[0:1, 0:1])
            nc.scalar.activation(fkscol[h][64:128, 1:2], fkc, AF.Identity,
                                 scale=vr_scale)
            for ncx in range(NC):
                cs = slice(ncx * 512, (ncx + 1) * 512)
                nv = PSU.tile([2, 512], f32, tag="big")
                nc.tensor.matmul(nv, qcsel_sb, QST[h][0:64, cs],
                                 start=True, stop=False)
                nc.tensor.matmul(nv, fkscol[h][64:128, 0:2],
                                 QST[h][64:128, cs], start=False, stop=True)
                nc.scalar.activation(row2(NVt, NVb, h)[:, cs], nv,
                                     AF.Identity, bias=vwcol_sb[:, 0:1])

        # ======== stage G: k-summaries (no N x N scores needed — the
        # bilinear form re-associates: out = (QST^T KST) fv = QST^T (KST fv))
        for h in range(HPG):
            hs = slice(h * 64, (h + 1) * 64)
            graw = PSS.tile([64, DIM_HEAD], f32, tag="pss")
            gnrm = PSS.tile([64, DIM_HEAD], f32, tag="pss")
            for mt in range(NT):
                nc.tensor.matmul(graw, fkm[mt][:, hs], fv_sb[mt][:, hs],
                                 start=(mt == 0), stop=(mt == NT - 1))
            for mt in range(NT):
                nc.tensor.matmul(gnrm, fknm[mt][:, hs], fv_sb[mt][:, hs],
                                 start=(mt == 0), stop=(mt == NT - 1))
            nc.vector.tensor_copy(G_sb[h][0:64, :], graw)
            nc.scalar.activation(G_sb[h][64:128, :], gnrm, AF.Copy)
            # WAB rows: wA = colsum(G_raw), wB = colsum(fv) — one psum pair
            wab = PSU.tile([2, DIM_HEAD], f32, tag="big")
            nc.tensor.matmul(wab, oc2_sb[0:64, 0:2], G_sb[h][0:64, :],
                             start=True, stop=False)
            for mt in range(NT):
                nc.tensor.matmul(wab, oc2_sb[:, 1:3], fv_sb[mt][:, hs],
                                 start=False, stop=(mt == NT - 1))
            nc.vector.tensor_copy(row2(WABt, WABb, h), wab)

        # ======== stage D: tiny out-stage + E ========
        for ncx in range(NC):
            cs = slice(ncx * 512, (ncx + 1) * 512)
            for h in range(HPG):
                hp, ds = h // 2, (h % 2) * 64
                po = PSU.tile([64, 512], f32, tag="big")
                nc.tensor.matmul(po, G_sb[h], QST[h][:, cs],
                                 start=True, stop=False)
                wabs = (WABt[32 * h:32 * h + 2, :] if h < 3 else WABb[0:2, :])
                nc.tensor.matmul(po, wabs, row2(NVt, NVb, h)[:, cs],
                                 start=False, stop=True)
                nc.scalar.activation(oT2[hp][ds:ds + 64, cs], po, AF.Copy)
            for nt in range(ncx * 4, ncx * 4 + 4):
                pf = PSU.tile([128, 512], f32, tag="big")
                for j in range(2):
                    nc.tensor.matmul(
                        pf, oT2[j][:, nt * 128:(nt + 1) * 128], wo_sb[j],
                        start=(j == 0), stop=(j == 1))
                ob = OSB.tile([128, 512], bf16, tag="ob")
                if nt % 2 == 0:
                    nc.scalar.activation(ob, pf, AF.Copy)
                else:
                    nc.vector.tensor_copy(ob, pf)
                nc.sync.dma_start(out=out[nt * 128:(nt + 1) * 128, :], in_=ob)

    _lp.__exit__(None, None, None)
    nc.compile()
    return nc


def _prep(q, k, v, ln_g, ln_b, W_in, W_out, b_out, cov_w_raw, var_w_raw):
    import ml_dtypes
    bf = ml_dtypes.bfloat16

    q = np.asarray(q, np.float32)
    k = np.asarray(k, np.float32)
    v = np.asarray(v, np.float32)
    ln_g = np.asarray(ln_g, np.float32)
    ln_b = np.asarray(ln_b, np.float32)
    W_in = np.asarray(W_in, np.float32)
    W_out = np.asarray(W_out, np.float32)

    cov_w = float(1.0 / (1.0 + np.exp(-np.float64(cov_w_raw))))
    var_w = float(1.0 / (1.0 + np.exp(-np.float64(var_w_raw))))
    cos_w = 1.0 - cov_w - var_w

    nc = _build_nc(cos_w, cov_w, var_w)

    W_f = (ln_g[:, None] * W_in).astype(np.float32)
    bW = (ln_b @ W_in).astype(np.float32)
    ident = np.eye(128, dtype=np.float32)
    sel2 = np.zeros((128, 2), np.float32)
    sel2[:64, 0] = 1.0
    sel2[64:, 1] = 1.0
    ocol = np.ones((128, 1), np.float32)
    oc2 = np.zeros((128, 3), np.float32)
    oc2[:, 0] = 1.0
    oc2[:, 2] = 1.0
    qcsel = np.zeros((64, 2), np.float32)
    qcsel[:, 0] = -1.0 / DIM_HEAD
    vwcol = np.zeros((2, 1), np.float32)
    vwcol[1, 0] = var_w
    e1v = np.zeros((64, 128), np.float32)
    e1v[0, :] = 1.0
    e2t = np.zeros((66, 128), np.float32)
    for h in range(3):
        e2t[32 * h, :64] = cos_w
        e2t[32 * h + 1, 64:] = 1.0
    e2b = np.zeros((2, 128), np.float32)
    e2b[0, :64] = cos_w
    e2b[1, 64:] = 1.0

    in_maps = []
    for core in range(8):
        b, g = core // HG, core % HG
        bWg = bW[g * IG:(g + 1) * IG]
        in_maps.append({
            "xq": np.ascontiguousarray(q[b]),
            "xk": np.ascontiguousarray(k[b]),
            "xv": np.ascontiguousarray(v[b]),
            "wf": np.ascontiguousarray(
                W_f[:, g * IG:(g + 1) * IG]).astype(bf),
            "wo": np.ascontiguousarray(
                W_out[g * IG:(g + 1) * IG, :]).astype(bf),
            "bwq": np.ascontiguousarray(
                bWg.reshape(HPG, 64).T * (cov_w / DIM_HEAD)).astype(
                    np.float32),
            "bwk": np.ascontiguousarray(
                bWg.reshape(HPG, 64).T).astype(np.float32),
            "bwv": np.concatenate(
                [bWg[None, :], np.zeros((63, IG), np.float32)],
                axis=0).astype(bf),
            "e1v": e1v.astype(bf),
            "ident": ident.astype(bf),
            "sel2": sel2.astype(bf),
            "ocol": ocol.astype(bf),
            "oc2": oc2.astype(bf),
            "qcsel": qcsel.astype(bf),
            "vwcol": vwcol,
            "e2t": e2t.astype(bf),
            "e2b": e2b.astype(bf),
        })
    return nc, in_maps


def kernel(q, k, v, ln_g, ln_b, W_in, W_out, b_out, cov_w_raw, var_w_raw):
    from concourse.bass_utils import run_bass_kernel_spmd

    b_out = np.asarray(b_out, np.float32)
    nc, in_maps = _prep(q, k, v, ln_g, ln_b, W_in, W_out, b_out,
                        cov_w_raw, var_w_raw)
    res = run_bass_kernel_spmd(nc, in_maps, list(range(8)))
    parts = [np.asarray(res.results[c]["out"], np.float32) for c in range(8)]
    out = np.stack([parts[2 * b] + parts[2 * b + 1] + b_out
                    for b in range(B)])
    return out.astype(np.float32)
